# revision 53
# baseline (speedup 1.0000x reference)
"""Trainium2 Bass kernel for per-head attention (TransformerLens-style).

Reference computation (per batch b, head h, with x = resid[b, :, h, :]):
    q = x @ W_Q[h] + b_Q[h];  k = x @ W_K[h] + b_K[h];  v = x @ W_V[h] + b_V[h]
    scores = q @ k.T / sqrt(DH), causal-masked, softmax over keys
    z = P @ v;  out[b, :, h, :] = z @ W_O[h] + b_O / H

Shapes: B=4, S=1024, H=12, DM=768, DH=64.  B*H = 48 independent attention
problems; 8 NeuronCores get 6 each (pure data parallel, no collectives).

v2 design notes (on top of the v1 couple/strip scheme -- see
kernel_v1_backup.py for the original docstring).  Measured best:
~164.6us (v1 baseline 164.6-166.5us).

  - LDWEIGHTS merging: bacc lowers every matmul to Ldweights+Matmult and a
    load costs ~107ns regardless of row count (cost scales with COLUMNS,
    streamed at 1.2 GHz independent of HAM).  _optimize_ldweights runs on
    the final SCHEDULED stream (the Tile scheduler reorders emission, so
    merge decisions must happen post-schedule): (1) row-merge two 64-row
    loads of vertical halves of one tile / col-merge two 64-col loads of
    one 128-col chunk into a single load (APs are mutated in place;
    same-semaphore ge-imm waits collapse to the max value, ISA wait-slot
    cap is 2); (2) cover-dedup repeats whose array region already holds
    identical data, forwarding their waits to the next PE instruction.
    Layouts built for (1): zT2[0:64]=even z^T strips / [64:128]=odd strips
    at the same columns (out-proj pairs), vT full-column transpose loads
    (the transpose stationary IS the data), host-side-duplicated [wv|wv]
    chunks (v-proj column-tile pair).  480 -> 334 loads.
  - l (softmax denominators): the z-psum row 64 (ones-column augmented v)
    is copied per-half into a [1, S] staging row, one gpsimd DMA scatters
    4 strips into partitions of an [8,128] lf tile (shape-mismatched
    SBUF-SBUF DMA: [1,512]->[4,128] is accepted and scatters by element
    order; DVE CANNOT write non-32-aligned partition bases), and ONE
    matmul against ident[0:8,0:8] transposes all 8 rows -> [128, 8]
    reciprocal (v1 used 8 ldweights+matmuls per pair for this).
  - PSUM 1-bank granules: "acc" ring bufs=4 rotates qk0lo,qk0hi,qk1lo,
    qk1hi,z0lo,z1lo,z0hi,z1hi per couple; z lo/hi split frees the lo bank
    mid-phase-B so the next couple's qk starts without waiting the z
    drain (v1's couple-boundary stalls ~1us).  vt/vtr/score share "scps"
    (bufs=2), out chunks + l + warmup share "ops" (bufs=2).  8 banks.
  - zT2 extraction: one strided-gather DVE copy per half (even strips ->
    top, odd -> bottom via the legal 32-aligned 0:64->64:128 partition
    shift); per-iteration on the last couple (lag-1 for the tail).
  - Cold start: ident loads first; ~40 dummy ident matmuls emitted before
    the qk loop warm the HAM clock-gate (4/8=1.2GHz default, releases to
    8/8=2.4GHz after ~3.4us sustained busy) during the DMA-bound ramp;
    couple-0 x loads split across sync(pair0)/gpsimd(pair1) queues and the
    bulk wqkv piece rides gpsimd so x chunks are never queued behind it.
  - Out-copies 5:3 DVE:Sc round-robin (ScalarE also carries all exps;
    measured Sc 69us / DVE 66us busy).

HAM/throttle reality measured across 11 HW runs: warm (K=8/8) fraction
rose 26% -> 33% as structure improved, so it IS schedule-coupled (not a
hard power cap), but the dominant pattern is: a ~1us PE gap at each
couple boundary re-throttles the clock-gate, and the ENTIRE next
projection phase (~20us) then runs at 1.2 GHz until phase B's density
releases it again.  Boundary gap cause: the next couple's qk matmuls
need acc-ring granules freed by the previous couple's z-hi extraction
(DVE) which competes with out-copies right at the boundary.  A pair-1
qk-stagger and moving the m=3 extraction to ScalarE were both tried and
MEASURED WORSE (+5us: ScalarE is not actually free at m=3, and the
stagger's stationary reloads cost more than the slack buys); the
granule-ring release order is what keeps the boundary gap at ~1us.
Engine floors: ScalarE
~70us (exps ~44us of it, per-ACTIVATE overhead (N+352)/1.2 ns), DVE
~67us, PE active ~130us at the throttled clock mix; MM concurrency
checks out (~40% of MM union at depth>=2 = the packable share; qk and z
are legitimately depth-1).

Dead ends measured on HW (do not retry blindly): fp8e4 DoubleRow matmuls
stream 1 output column/cycle (not the cost model's 0.5), so a hi+lo fp8
split costs 1.5x bf16; DoubleRow also cannot write a column-packed psum
dst at partition base 64 (ISA s3d3_mm_valid_dst_partition).  dma_start
rejects PSUM APs (no direct psum->DRAM store; all psum drains go through
Sc/DVE).  Scattering >70 warm-filler matmuls through the stream ADDS
~10us (they run mostly cold).  Strided-gather gpsimd DMAs (kkT-style
column compression) cost ~4x contiguous in DGE descriptor time and
clogged the gpsimd queue (+22us busy) -- keep gpsimd DMAs contiguous.
MM output psum dst must sit within ONE 2KB bank (512 f32) -- merged-exp
[128,1024] score tiles would need 4 scps banks (psum budget is exactly
8: acc 4 + scps 2 + ops 2).  ops bufs=1 fails walrus codegen (an MM ends
up with >2 sync waits: S3D3_MM wait-slot cap).  Gating the v-projection
behind qkT copies via acc-ring granules costs more (PE hole at the qk->v
transition) than the ~50 interleave-reloads it saves.  z col-tiling
across the couple's pairs (the remaining 2x on the z phase) is closed:
any l computation requires a second pt stream through the PE -- the only
free ride is the 65th stationary column, which is exactly what blocks
the packing.
"""

import os
import numpy as np
import ml_dtypes
from contextlib import ExitStack

B, S, H, DM, DH = 4, 1024, 12, 768, 64
N_CORES = 8
PAIRS = B * H
PPC = PAIRS // N_CORES      # pairs per core
CPC = PPC // 2              # couples per core

BF16 = ml_dtypes.bfloat16

LAST_EXEC_TIME_NS = None
LAST_RESULTS = None


def _core_pair_map():
    """(b, h) for each (core, slot).  Couple g = (head g//2, batch-half g%2);
    core c owns couples 3c..3c+2, slot s -> couple 3c + s//2, e = s%2."""
    m = []
    for c in range(N_CORES):
        row = []
        for s in range(PPC):
            g = 3 * c + s // 2
            h, bh, e = g // 2, g % 2, s % 2
            row.append((2 * bh + e, h))
        m.append(row)
    return m


def _strip_blocks(i, s_len):
    """128-aligned score blocks for strip i: start at the diagonal."""
    v = 128 * i
    if v < 512:
        return [(v, 512), (512, s_len)]
    return [(v, s_len)]


def build_nc(n_couples=CPC, s_len=S, dm=DM, dh=DH):
    import concourse.bacc as bacc
    import concourse.tile as tile
    import concourse.mybir as mybir

    f32 = mybir.dt.float32
    bf16 = mybir.dt.bfloat16
    KC = dm // 128
    NSQ = s_len // 128
    NM = NSQ // 2            # strip-pairs
    MMB = 512

    nc = bacc.Bacc("TRN2", target_bir_lowering=False, debug=False)

    WQK = KC * 2 * dh        # qk weight region columns
    WVW = KC * 128           # v weight region columns (wv duplicated)
    xt = nc.declare_dram_parameter("xt", [2 * n_couples, 128, KC * s_len], bf16, isOutput=False)
    wqkv = nc.declare_dram_parameter("wqkv", [n_couples, 128, WQK + WVW], bf16, isOutput=False)
    wo = nc.declare_dram_parameter("wo", [n_couples, 128, dm], bf16, isOutput=False)
    ident = nc.declare_dram_parameter("ident", [128, 128], bf16, isOutput=False)
    out = nc.declare_dram_parameter("out", [2 * n_couples, NM, 128, 2 * dm], bf16, isOutput=True)

    Exp = mybir.ActivationFunctionType.Exp
    WVO = WQK  # column offset of wv within wqkv

    with ExitStack() as ctx:
        tc = ctx.enter_context(tile.TileContext(nc))

        xt_pool = ctx.enter_context(tc.tile_pool(name="xt", bufs=2 * n_couples))
        wqkv_pool = ctx.enter_context(tc.tile_pool(name="wqkv", bufs=n_couples))
        wo_pool = ctx.enter_context(tc.tile_pool(name="wo", bufs=n_couples))
        const_pool = ctx.enter_context(tc.tile_pool(name="const", bufs=1))
        qkT_pool = ctx.enter_context(tc.tile_pool(name="qkT", bufs=3))
        qdup_pool = ctx.enter_context(tc.tile_pool(name="qdup", bufs=3))
        vT_pool = ctx.enter_context(tc.tile_pool(name="vT", bufs=2))
        vaug_pool = ctx.enter_context(tc.tile_pool(name="vaug", bufs=2))
        pstrip_pool = ctx.enter_context(tc.tile_pool(name="pstrip", bufs=12))
        zT2_pool = ctx.enter_context(tc.tile_pool(name="zT2", bufs=4))
        lf_pool = ctx.enter_context(tc.tile_pool(name="lf", bufs=4))
        stage_pool = ctx.enter_context(tc.tile_pool(name="stage", bufs=4))
        recip_pool = ctx.enter_context(tc.tile_pool(name="recip", bufs=4))
        osb_pool = ctx.enter_context(tc.tile_pool(name="osb", bufs=8))

        # PSUM (8 banks): acc = 4x 1-bank granules rotating
        # qk0lo,qk0hi,qk1lo,qk1hi,z0lo,z1lo,z0hi,z1hi per couple;
        # scps = vt granules + v-transposes + score blocks (2 banks);
        # ops = out-proj chunks + l columns (2 banks).
        acc_pool = ctx.enter_context(tc.tile_pool(name="acc", bufs=4, space="PSUM"))
        scps = ctx.enter_context(tc.tile_pool(name="scps", bufs=2, space="PSUM"))
        ops_pool = ctx.enter_context(tc.tile_pool(name="ops", bufs=2, space="PSUM"))

        # ---- loads are issued just-in-time, one couple ahead ----
        wqkv_sbs, wo_sbs, x_sbs = [], [], []
        kh = KC // 2

        def issue_couple_loads(g, fine):
            wqkv_sb = wqkv_pool.tile([128, WQK + WVW], bf16, name=f"wqkv_{g}", tag="wqkv")
            wo_sb = wo_pool.tile([128, dm], bf16, name=f"wo_{g}", tag="wo")
            if fine:
                # first couple: per-chunk pieces in consumption order so the
                # qk kc-loop never outruns the load stream
                nc.sync.dma_start(wqkv_sb[:, :2 * 2 * dh], wqkv[g, :, :2 * 2 * dh])
                xts = []
                for e in (0, 1):
                    p = 2 * g + e
                    xtile = xt_pool.tile([128, KC * s_len], bf16, name=f"x_{p}", tag="x")
                    xts.append(xtile)
                    x_sbs.append(xtile)
                # pair 0 on the sync queue, pair 1 on the gpsimd queue --
                # two DMA channels halve the cold-start load time
                XQ = {0: nc.sync, 1: nc.gpsimd}
                for e in (0, 1):
                    XQ[e].dma_start(xts[e][:, :512], xt[2 * g + e, :, :512])
                # bulk weight piece rides the gpsimd queue so it doesn't
                # delay pair-0's x chunks on the sync queue
                nc.gpsimd.dma_start(wqkv_sb[:, 2 * 2 * dh:], wqkv[g, :, 2 * 2 * dh:])
                for e in (0, 1):
                    XQ[e].dma_start(xts[e][:, 512:s_len], xt[2 * g + e, :, 512:s_len])
                for kc in range(1, KC):
                    for e in (0, 1):
                        XQ[e].dma_start(
                            xts[e][:, kc * s_len:(kc + 1) * s_len],
                            xt[2 * g + e, :, kc * s_len:(kc + 1) * s_len])
                nc.sync.dma_start(wo_sb[:], wo[g])
            else:
                nc.sync.dma_start(wqkv_sb[:], wqkv[g])
                for e in (0, 1):
                    p = 2 * g + e
                    xtile = xt_pool.tile([128, KC * s_len], bf16, name=f"x_{p}", tag="x")
                    nc.sync.dma_start(xtile[:, :kh * s_len], xt[p, :, :kh * s_len])
                    nc.sync.dma_start(xtile[:, kh * s_len:], xt[p, :, kh * s_len:])
                    x_sbs.append(xtile)
                nc.sync.dma_start(wo_sb[:], wo[g])
            wqkv_sbs.append(wqkv_sb)
            wo_sbs.append(wo_sb)

        # ident loads FIRST (tiny) so the HAM warm-up burst below can start
        # immediately; the real loads follow on the same queue
        ident_sb = const_pool.tile([128, 128], bf16, name="ident_sb")
        nc.sync.dma_start(ident_sb[:], ident[:, :])
        issue_couple_loads(0, fine=True)
        if n_couples > 1:
            issue_couple_loads(1, fine=False)

        # HAM warm-up/keep-warm fillers: the PE clock-gate defaults to 4/8
        # (1.2 GHz) and only releases to 8/8 after ~3.4us of sustained busy;
        # any ~us idle re-throttles.  The first couple is DMA-load-bound, so
        # dummy ident matmuls are sprinkled at priorities BETWEEN the real
        # work: the list scheduler only runs them when nothing else is ready,
        # keeping the PE array busy through load stalls.
        wu_ps = ops_pool.tile([128, 128], f32, name="warm_ps", tag="ops")

        def warm_fill(n):
            for _ in range(n):
                nc.tensor.matmul(wu_ps[:, 0:128], lhsT=ident_sb[:, :],
                                 rhs=ident_sb[:, :], start=True, stop=True,
                                 skip_group_check=True)

        # 60 matmuls x ~107ns bridges from DMA-queue boot (~3us) to the
        # first data-ready qk matmuls (~9.5us) with no idle window between
        warm_fill(60)

        # engine round-robin for out-copies (psum readers: DVE/ScalarE only);
        # 5:3 DVE:Sc because ScalarE also carries all the exps
        OUT_ENGS = [nc.vector, nc.scalar, nc.vector, nc.scalar,
                    nc.vector, nc.vector, nc.scalar, nc.vector]
        out_rr = [0]

        pending_out = []
        final_split = [False]

        def emit_one_pending():
            if pending_out:
                pending_out.pop(0)()

        def make_out(p, j, zT2_sb, recip_sb, wo_sb):
            """Out-projection for strip-couple (j, j+1): one merged 128-row
            ldweights of zT2 block m, row-packed dj matmuls."""
            m = j // 2
            e = p & 1

            def emit():
                o_sb = osb_pool.tile([128, 2 * dm], bf16, name=f"osb_{p}_{j}", tag="osb")
                for c0 in range(0, dm, MMB):
                    c1 = min(c0 + MMB, dm)
                    o_tiles = []
                    for dj in (0, 1):
                        o_ps = ops_pool.tile([128, 512], f32, name=f"ops_{p}_{j + dj}_{c0}", tag="ops")
                        nc.tensor.matmul(
                            o_ps[:, 0:c1 - c0],
                            lhsT=zT2_sb[64 * dj:64 * dj + dh,
                                        m * 128:(m + 1) * 128],
                            rhs=wo_sb[64 * dj:64 * dj + dh, c0:c1],
                            start=True, stop=True,
                        )
                        o_tiles.append(o_ps)
                    for dj in (0, 1):
                        dst = o_sb[:, dj * dm + c0:dj * dm + c1]
                        osrc = o_tiles[dj][:, 0:c1 - c0]
                        scal = recip_sb[:, e * 4 + (j + dj) % 4:e * 4 + (j + dj) % 4 + 1]
                        eng = OUT_ENGS[out_rr[0] % len(OUT_ENGS)]
                        out_rr[0] += 1
                        if eng is nc.scalar:
                            nc.scalar.mul(dst, osrc, scal)
                        else:
                            eng.tensor_scalar_mul(dst, osrc, scal)
                if final_split[0]:
                    # tail: halve store latency across two queues
                    nc.sync.dma_start(out[p, m][:, :dm], o_sb[:, :dm])
                    nc.gpsimd.dma_start(out[p, m][:, dm:], o_sb[:, dm:])
                else:
                    nc.sync.dma_start(out[p, m], o_sb[:])
            return emit

        for g in range(n_couples):
            if g + 2 < n_couples:
                issue_couple_loads(g + 2, fine=False)
            p0, p1 = 2 * g, 2 * g + 1
            x0, x1 = x_sbs[p0], x_sbs[p1]
            wqkv_sb = wqkv_sbs[g]
            wo_sb = wo_sbs[g]
            last = g == n_couples - 1

            # ---- qk^T projections into 1-bank granules, shared stationary ----
            qk_gr = {}
            for e in (0, 1):
                for hh in (0, 1):
                    qk_gr[e, hh] = acc_pool.tile(
                        [128, 512], f32, name=f"qkps_{2 * g + e}_{hh}", tag="acc")
            # kc-outer with pair 1 staggered by two kc chunks: the couple's
            # first four matmuls touch only pair 0's granules (ring slots
            # freed at the previous couple's m=1), so the boundary matmuls
            # need not wait the z-hi extraction (costs 2 extra ldweights)
            qk_seq = [(0, 0), (1, 0), (0, 1), (1, 1)]
            qk_seq += [(kc, e) for kc in range(2, KC) for e in (0, 1)]
            for kc, e in qk_seq:
                xtile = x0 if e == 0 else x1
                for hh in (0, 1):
                    n0 = hh * 512
                    nc.tensor.matmul(
                        qk_gr[e, hh][:, 0:512],
                        lhsT=wqkv_sb[:, kc * 2 * dh:(kc + 1) * 2 * dh],
                        rhs=xtile[:, kc * s_len + n0:kc * s_len + n0 + 512],
                        start=(kc == 0), stop=(kc == KC - 1),
                        skip_group_check=(e == 1 or hh == 1),
                    )
            qkTs, qdups = [], []
            for e, p in ((0, p0), (1, p1)):
                qkT_sb = qkT_pool.tile([128, s_len], bf16, name=f"qkT_{p}", tag="qkT")
                # lo/hi on different engines so both copies run concurrently
                if e == 0:
                    nc.scalar.copy(qkT_sb[:, 0:512], qk_gr[e, 0][:, 0:512])
                    nc.vector.tensor_copy(qkT_sb[:, 512:1024], qk_gr[e, 1][:, 0:512])
                else:
                    nc.vector.tensor_copy(qkT_sb[:, 0:512], qk_gr[e, 0][:, 0:512])
                    nc.scalar.copy(qkT_sb[:, 512:1024], qk_gr[e, 1][:, 0:512])
                # partition swap: swap[0:64]=k^T, swap[64:128]=q^T -- two
                # contiguous [64,1024] DMAs (cheap descriptor count)
                swap_sb = qdup_pool.tile([128, s_len], bf16, name=f"swap_{p}", tag="qdup")
                nc.gpsimd.dma_start(swap_sb[0:dh, :], qkT_sb[dh:2 * dh, :])
                nc.gpsimd.dma_start(swap_sb[dh:2 * dh, :], qkT_sb[0:dh, :])
                qkTs.append(qkT_sb)
                qdups.append(swap_sb)
                emit_one_pending()

            # ---- v^T projections, column-packed via duplicated weights ----
            vt_gr = [scps.tile([128, 512], f32, name=f"vtps_{g}_{hh}", tag="scps")
                     for hh in (0, 1)]
            for kc in range(KC):
                for hh in (0, 1):
                    n0 = hh * 512
                    for e, xtile in ((0, x0), (1, x1)):
                        nc.tensor.matmul(
                            vt_gr[hh][64 * e:64 * e + dh, 0:512],
                            lhsT=wqkv_sb[:, WVO + kc * 128 + 64 * e:
                                         WVO + kc * 128 + 64 * e + dh],
                            rhs=xtile[:, kc * s_len + n0:kc * s_len + n0 + 512],
                            start=(kc == 0), stop=(kc == KC - 1),
                            skip_group_check=True,
                        )
            vT_sb = vT_pool.tile([128, s_len], bf16, name=f"vT_{g}", tag="vT")
            nc.vector.tensor_copy(vT_sb[:, 0:512], vt_gr[0][:, 0:512])
            nc.scalar.copy(vT_sb[:, 512:], vt_gr[1][:, 0:512])
            emit_one_pending()
            emit_one_pending()

            # bf16 transposes: one merged 128-row transpose-load per t-block
            vtrs = [scps.tile([128, NSQ * dh], bf16, name=f"vtr_{2 * g + e}", tag="scps")
                    for e in (0, 1)]
            for t in range(NSQ):
                for e in (0, 1):
                    nc.tensor.transpose(
                        vtrs[e][:, t * dh:(t + 1) * dh],
                        vT_sb[64 * e:64 * e + dh, t * 128:(t + 1) * 128],
                        ident_sb[64 * e:64 * e + dh, 64 * e:64 * e + dh],
                    )
            vaugs = []
            for e, p in ((0, p0), (1, p1)):
                vaug_sb = vaug_pool.tile([128, NSQ * (dh + 1)], bf16, name=f"vaug_{p}", tag="vaug")
                if g == 0:
                    # ones columns persist across pool reuse; set once
                    nc.gpsimd.memset(vaug_sb[:], 1.0)
                nc.vector.tensor_copy(
                    vaug_sb[:].rearrange("p (n d) -> p n d", d=dh + 1)[:, :, 0:dh],
                    vtrs[e][:].rearrange("p (n d) -> p n d", d=dh),
                )
                vaugs.append(vaug_sb)

            # ---- phase B: pairs interleaved per strip-pair ----
            z_gr, zT2_sbs = {}, {}
            lf_sbs = {}
            recip_sbs = {}
            for e, p in ((0, p0), (1, p1)):
                zT2_sbs[e] = zT2_pool.tile([128, NM * 128], bf16, name=f"zT2_{p}", tag="zT2")
            # z granules allocated in release-friendly order
            for e, p in ((0, p0), (1, p1)):
                z_gr[e, 0] = acc_pool.tile([128, 512], f32, name=f"zps_{p}_lo", tag="acc")
            for e, p in ((0, p0), (1, p1)):
                z_gr[e, 1] = acc_pool.tile([128, 512], f32, name=f"zps_{p}_hi", tag="acc")
            for hh in (0, 1):
                lf_sbs[hh] = lf_pool.tile([8, 128], bf16, name=f"lf_{g}_{hh}", tag="lf")
                recip_sbs[hh] = recip_pool.tile([128, 8], f32, name=f"recip_{g}_{hh}", tag="recip")
            stage_sbs = {e: stage_pool.tile([1, s_len], bf16, name=f"lstage_{2 * g + e}", tag="stage")
                         for e in (0, 1)}

            for m in range(NM):
                i0 = 2 * m
                blocks0 = _strip_blocks(i0, s_len)
                blocks1 = _strip_blocks(i0 + 1, s_len)
                nblk = max(len(blocks0), len(blocks1))
                sc_tiles = {}
                # row-packed score matmuls for BOTH pairs (k^T strips as
                # stationaries in opposite partition halves)
                for e, p in ((0, p0), (1, p1)):
                    qkT_sb, swap_sb = qkTs[e], qdups[e]
                    for bi in range(nblk):
                        for di, i, blocks in ((0, i0, blocks0), (1, i0 + 1, blocks1)):
                            bj = bi - (nblk - len(blocks))
                            if bj < 0:
                                continue
                            a, b = blocks[bj]
                            sc_ps = scps.tile([128, 512], f32, name=f"sc_{p}_{i}_{a}", tag="scps")
                            if di == 0:
                                lhsT = swap_sb[0:dh, i * 128:(i + 1) * 128]
                                rhs = qkT_sb[0:dh, a:b]
                            else:
                                lhsT = qkT_sb[dh:2 * dh, i * 128:(i + 1) * 128]
                                rhs = swap_sb[dh:2 * dh, a:b]
                            nc.tensor.matmul(
                                sc_ps[:, 0:b - a], lhsT=lhsT, rhs=rhs,
                                start=True, stop=True,
                            )
                            sc_tiles[(e, i, a)] = sc_ps

                # PE gap fillers: deferred out-couples run here
                emit_one_pending()
                emit_one_pending()

                # exp (ScalarE), diag mask (GpSimd), z matmuls, extraction
                for e, p in ((0, p0), (1, p1)):
                    vaug_sb = vaugs[e]
                    zT2_sb = zT2_sbs[e]
                    for di, i, blocks in ((0, i0, blocks0), (1, i0 + 1, blocks1)):
                        # all exps of the strip first, then both z matmuls
                        # back-to-back so the vaug stationary loads once
                        pts = []
                        for (a, b) in blocks:
                            sc_ps = sc_tiles[(e, i, a)]
                            pt_sb = pstrip_pool.tile([128, 512], bf16, name=f"pt_{p}_{i}_{a}", tag="pstrip")
                            nc.scalar.activation(pt_sb[:, 0:b - a], sc_ps[:, 0:b - a], Exp)
                            if a == 128 * i:  # leading block holds the diag triangle
                                dst = pt_sb[:, 0:128]
                                nc.gpsimd.affine_select(
                                    out=dst, in_=dst,
                                    compare_op=mybir.AluOpType.is_ge,
                                    fill=0.0, base=0,
                                    pattern=[[1, 128]], channel_multiplier=-1,
                                )
                            pts.append(pt_sb)
                        for (a, b), pt_sb in zip(blocks, pts):
                            # z dst granule(s): blocks never straddle col 512
                            gr = z_gr[e, 0] if b <= 512 else z_gr[e, 1]
                            goff = 0 if b <= 512 else 512
                            nc.tensor.matmul(
                                gr[0:dh + 1, a - goff:b - goff],
                                lhsT=vaug_sb[:, i * (dh + 1):(i + 1) * (dh + 1)],
                                rhs=pt_sb[:, 0:b - a],
                                start=(i == 0), stop=(i == (b - 1) // 128),
                                skip_group_check=True,
                            )

                    # eager extraction: even strips -> zT2 top half, odd ->
                    # bottom (DVE psum partitions 0:64 -> sbuf 64:128 is a
                    # legal 32-aligned shift).  Non-last couples defer to one
                    # strided-gather copy per half (half the instruction
                    # overhead); the last couple stays per-iteration (lag 1).
                    hh = m // 2
                    gr = z_gr[e, 0] if m < 2 else z_gr[e, 1]
                    c0 = 256 * m - (0 if m < 2 else 512)
                    if last:
                        nc.vector.tensor_copy(
                            zT2_sb[0:dh, m * 128:(m + 1) * 128], gr[0:dh, c0:c0 + 128])
                        nc.vector.tensor_copy(
                            zT2_sb[dh:2 * dh, m * 128:(m + 1) * 128], gr[0:dh, c0 + 128:c0 + 256])
                        nc.vector.tensor_copy(
                            stage_sbs[e][0:1, 256 * m:256 * m + 256], gr[dh:dh + 1, c0:c0 + 256])
                    elif m in (1, 3):
                        hv = gr[:, 0:512].rearrange("p (t o b) -> p t o b", o=2, b=128)
                        zv = zT2_sb[:, (m - 1) * 128:(m + 1) * 128].rearrange(
                            "p (t b) -> p t b", b=128)
                        nc.vector.tensor_copy(zv[0:dh], hv[0:dh, :, 0, :])
                        nc.vector.tensor_copy(zv[dh:2 * dh], hv[0:dh, :, 1, :])
                        nc.vector.tensor_copy(
                            stage_sbs[e][0:1, 512 * hh:512 * (hh + 1)],
                            gr[dh:dh + 1, 0:512])

                # l transpose: one 8-row matmul per couple-half gives both
                # pairs' denominator columns; last couple runs it per
                # iteration (lag-1) to shrink the final flush
                if m in (1, 3) or last:
                    hh = m // 2
                    lf_sb, recip_sb = lf_sbs[hh], recip_sbs[hh]
                    for e in (0, 1):
                        if last:
                            r0 = e * 4 + (2 * m) % 4
                            nc.gpsimd.dma_start(
                                lf_sb[r0:r0 + 2, 0:128],
                                stage_sbs[e][0:1, 256 * m:256 * (m + 1)])
                        else:
                            nc.gpsimd.dma_start(
                                lf_sb[e * 4:e * 4 + 4, 0:128],
                                stage_sbs[e][0:1, 512 * hh:512 * (hh + 1)])
                    l_ps = ops_pool.tile([128, 8], f32, name=f"lps_{g}_{m}", tag="ops")
                    nc.tensor.matmul(
                        l_ps[:, 0:8], lhsT=lf_sb[0:8, 0:128],
                        rhs=ident_sb[0:8, 0:8], start=True, stop=True,
                    )
                    nc.vector.reciprocal(recip_sb[:, 0:8], l_ps[:, 0:8])
                    if last:
                        for e, p in ((0, p0), (1, p1)):
                            pending_out.append(make_out(p, 2 * m, zT2_sbs[e], recip_sb, wo_sb))
                    else:
                        for e, p in ((0, p0), (1, p1)):
                            pending_out.append(make_out(p, 2 * m - 2, zT2_sbs[e], recip_sb, wo_sb))
                            pending_out.append(make_out(p, 2 * m, zT2_sbs[e], recip_sb, wo_sb))

        final_split[0] = True
        while pending_out:
            emit_one_pending()

    nc.finalize()
    _optimize_ldweights(nc, mybir)
    return nc


def _optimize_ldweights(nc, mybir):
    """Merge and dedup PE Ldweights on the final scheduled stream.

    bacc lowers every matmul to an Ldweights+Matmult pair, walrus runs with
    --enable-ldw-opt=false, and ldweights streams at ~1.2 GHz with cost
    proportional to its COLUMN count only -- a 128-row load costs the same
    as a 64-row one.  Two rewrites:

    1. pair-merge: a row-packed pair loads vertical halves of the same tile
       at the same columns (kkT/zT2/vT layouts are built for this), and the
       duplicated-wv layout loads horizontal halves of one 128-col chunk.
       The earlier load's AP is widened to cover both and the later load is
       deleted (its waits join the merged load: both halves' producers must
       be done before the single load streams).
    2. cover-dedup: a load is dropped when the array region it would fill
       already holds identical data (repeat stationaries across moving
       blocks); its waits move onto the next PE instruction.

    Merging is restricted to a tensor-name safe-list whose producers never
    depend on PE progress between the pair (no wait-cycle risk)."""
    pe = mybir.EngineType.PE
    MERGE_OK = {"kkT", "zT2", "vT", "wqkv"}
    removed = 0

    def parse(inst):
        ap = inst.ins[0]
        a = [list(x) for x in ap.ap]
        if len(a) != 2 or a[1][0] != 1:
            return None
        (pstride, np_), (_, ncols) = a
        if pstride <= 0:
            return None
        pbase, coff = divmod(ap.offset, pstride)
        tp = inst.tile_position or (0, 0)
        return dict(mem=ap.memref, dt=repr(ap.dtype), np=np_, ncols=ncols,
                    pstride=pstride, pbase=pbase, coff=coff, tp=tp,
                    it=repr(inst.is_transpose), pm=repr(inst.perf_mode))

    def region(d):
        return (d["tp"][0], d["tp"][0] + d["np"], d["tp"][1], d["tp"][1] + d["ncols"])

    def overlaps(a, b):
        return a[0] < b[1] and b[0] < a[1] and a[2] < b[3] and b[2] < a[3]

    def covers(k, c):
        if (k["mem"] != c["mem"] or k["dt"] != c["dt"] or k["it"] != c["it"]
                or k["pm"] != c["pm"]):
            return False
        if k["tp"][0] - k["pbase"] != c["tp"][0] - c["pbase"]:
            return False
        if k["tp"][1] - k["coff"] != c["tp"][1] - c["coff"]:
            return False
        rk, rc = region(k), region(c)
        return rk[0] <= rc[0] and rc[1] <= rk[1] and rk[2] <= rc[2] and rc[3] <= rk[3]

    def same_meta(x, y):
        return (x["mem"] == y["mem"] and x["dt"] == y["dt"] and x["it"] == y["it"]
                and x["pm"] == y["pm"] and x["pstride"] == y["pstride"])

    def pair_kind(x, y):
        """row: vertical halves (top at (0,c), bottom at (64,c), same cols);
        col: horizontal halves of one 128-col chunk at (0,0)/(0,64)."""
        if not same_meta(x, y) or x["mem"].split("_")[0] not in MERGE_OK:
            return None
        if (x["np"] == 64 and y["np"] == 64 and x["ncols"] == y["ncols"]
                and x["coff"] == y["coff"] and x["tp"][1] == y["tp"][1]
                and {(x["pbase"], x["tp"][0]), (y["pbase"], y["tp"][0])}
                == {(0, 0), (64, 64)}):
            return "row"
        if (x["np"] == 128 and y["np"] == 128 and x["ncols"] == 64 and y["ncols"] == 64
                and x["pbase"] == 0 and y["pbase"] == 0
                and {(x["coff"] - min(x["coff"], y["coff"]), x["tp"][1]),
                     (y["coff"] - min(x["coff"], y["coff"]), y["tp"][1])}
                == {(0, 0), (64, 64)}):
            return "col"
        return None

    def apply_merge(inst, x, y, kind):
        ap = inst.ins[0]
        if kind == "row":
            ap.offset = x["coff"]
            ap.ap = [[x["pstride"], 128], [1, x["ncols"]]]
            inst.tile_position = (0, x["tp"][1])
            inst.tile_size = (128, x["ncols"])
            return dict(x, np=128, pbase=0, coff=x["coff"], tp=(0, x["tp"][1]))
        else:
            c0 = min(x["coff"], y["coff"])
            ap.offset = c0
            ap.ap = [[x["pstride"], 128], [1, 128]]
            inst.tile_position = (0, 0)
            inst.tile_size = (128, 128)
            return dict(x, ncols=128, coff=c0, tp=(0, 0))

    MAX_WAITS = 2

    def dedup_waits(waits):
        """Collapse same-semaphore ge-imm waits to the max value."""
        out = []
        best = {}
        for w in waits:
            if getattr(w, "wait_mode", None) == "sem-ge-imm" and w.wait_reg is None:
                key = (w.sync_type, w.id)
                cur = best.get(key)
                if cur is None:
                    best[key] = w
                    out.append(w)
                elif w.wait_value > cur.wait_value:
                    out[out.index(cur)] = w
                    best[key] = w
            else:
                out.append(w)
        return out

    def try_merge_sync(dst_inst, src_inst):
        """Combine src's waits/updates into dst; False if over the ISA cap."""
        ssi = src_inst.sync_info
        dsi = dst_inst.sync_info
        waits = dedup_waits(
            (list(dsi.on_wait) if dsi else []) + (list(ssi.on_wait) if ssi else []))
        upds = (list(dsi.on_update) if dsi else []) + (list(ssi.on_update) if ssi else [])
        if len(waits) > MAX_WAITS:
            return False
        if dsi is None:
            dst_inst.sync_info = mybir.SyncInfo(on_wait=waits, on_update=upds)
        else:
            dsi.on_wait = waits
            dsi.on_update = upds
        return True

    ES = getattr(mybir, "InstEventSemaphore", ())

    for fn in nc.m.functions:
        for blk in fn.blocks:
            insts = blk.instructions
            n = len(insts)
            pe_idx = [i for i in range(n)
                      if getattr(insts[i], "engine", None) == pe]
            drop = set()

            # ---- pass 1: pair-merge on the PE stream ----
            k = 0
            while k < len(pe_idx):
                i = pe_idx[k]
                inst = insts[i]
                if i in drop or not isinstance(inst, mybir.InstLdweights):
                    k += 1
                    continue
                x = parse(inst)
                if x is None or x["mem"].split("_")[0] not in MERGE_OK:
                    k += 1
                    continue
                # scan forward for the partner: MMs/event-sems may intervene
                j = k + 1
                partner = None
                mid_mms = []
                while j < len(pe_idx):
                    inst2 = insts[pe_idx[j]]
                    if isinstance(inst2, mybir.InstLdweights):
                        y = parse(inst2)
                        if y is not None and pair_kind(x, y):
                            partner = (pe_idx[j], inst2, y)
                        break
                    if isinstance(inst2, mybir.InstMatmult):
                        mid_mms.append(inst2)
                        j += 1
                        continue
                    if ES and isinstance(inst2, ES):
                        j += 1
                        continue
                    break
                if partner is None:
                    k += 1
                    continue
                jj, y_inst, y = partner
                # the partner's region gets loaded earlier than before: no
                # intervening matmul may be streaming through it
                yreg = region(y) if pair_kind(x, y) == "row" else (
                    0, 128, y["tp"][1], y["tp"][1] + y["ncols"])
                bad = False
                for mm in mid_mms:
                    tp = mm.tile_position or (0, 0)
                    ts = mm.tile_size or (128, 128)
                    if overlaps((tp[0], tp[0] + ts[0], tp[1], tp[1] + ts[1]), yreg):
                        bad = True
                        break
                if bad or not try_merge_sync(inst, y_inst):
                    k += 1
                    continue
                apply_merge(inst, x, y, pair_kind(x, y))
                drop.add(jj)
                removed += 1
                k += 1

            # ---- pass 2: cover-dedup; waits forward to the next PE inst ----
            live = [ii for ii in pe_idx if ii not in drop]
            kept = []
            for pos, ii in enumerate(live):
                inst = insts[ii]
                if isinstance(inst, mybir.InstLdweights):
                    d = parse(inst)
                    si = inst.sync_info
                    no_upd = si is None or not si.on_update
                    if d is not None and no_upd and any(covers(kk, d) for kk in kept):
                        ok = True
                        if si is not None and si.on_wait:
                            ok = (pos + 1 < len(live)
                                  and try_merge_sync(insts[live[pos + 1]], inst))
                        if ok:
                            removed += 1
                            drop.add(ii)
                            continue
                    if d is not None:
                        kept = [kk for kk in kept if not overlaps(region(kk), region(d))]
                        kept.append(d)
                    else:
                        kept = []
                elif isinstance(inst, mybir.InstMatmult):
                    pass
                elif ES and isinstance(inst, ES):
                    pass
                else:
                    kept = []

            if drop:
                keep = [insts[i] for i in range(n) if i not in drop]
                del blk.instructions[:]
                for inst in keep:
                    blk.instructions.append(inst)
    return removed


def prepare_shards(normalized_resid_pre, W_Q, b_Q, W_K, b_K, W_V, b_V, W_O, b_O):
    """Host-side layout: returns in_maps for the 8 cores."""
    x = np.asarray(normalized_resid_pre, dtype=np.float32)
    scale = 1.0 / np.sqrt(DH)
    KC = DM // 128

    pair_map = _core_pair_map()

    # x^T per (core, slot), partition-major: [128, KC*S]
    xt_f = x.transpose(0, 2, 3, 1)  # [B, H, DM, S]
    # W_Q pre-scaled by 1/sqrt(DH) so scores come out pre-scaled
    wqk_h = np.concatenate([np.asarray(W_Q) * scale, np.asarray(W_K)], axis=-1)
    wv_h = np.asarray(W_V)  # [H, DM, DH]
    wo_h = np.asarray(W_O)  # [H, DH, DM]

    ident = np.eye(128).astype(BF16)

    in_maps = []
    for c in range(N_CORES):
        xts, wqkvs, wos = [], [], []
        for s in range(PPC):
            b, h = pair_map[c][s]
            xts.append(
                xt_f[b, h].reshape(KC, 128, S).transpose(1, 0, 2).reshape(128, KC * S))
            if s % 2 == 0:
                wv_c = wv_h[h].reshape(KC, 128, DH).transpose(1, 0, 2)  # [128, KC, DH]
                wv_dup = np.concatenate([wv_c, wv_c], axis=2)  # [128, KC, 128]
                wqkvs.append(np.concatenate(
                    [wqk_h[h].reshape(KC, 128, 2 * DH).transpose(1, 0, 2).reshape(128, KC * 2 * DH),
                     wv_dup.reshape(128, KC * 128)],
                    axis=1))
                wos.append(np.concatenate([wo_h[h], wo_h[h]], axis=0))  # [128, DM]
        in_maps.append({
            "xt": np.ascontiguousarray(np.stack(xts)).astype(BF16),
            "wqkv": np.ascontiguousarray(np.stack(wqkvs)).astype(BF16),
            "wo": np.ascontiguousarray(np.stack(wos)).astype(BF16),
            "ident": ident,
        })
    return in_maps


def _ensure_profile_hook():
    """The agent image lacks ``antenv.axon_hooks``; shim it and install the
    ctypes NTFF hook from trn_boot so trace=True works under axon."""
    import importlib
    import sys
    import types
    try:
        importlib.import_module("antenv.axon_hooks")
        return True
    except ImportError:
        pass
    try:
        import antenv
        mod = types.ModuleType("antenv.axon_hooks")
        _state = {"hook": None}
        mod.set_axon_ntff_profile_hook = lambda h: _state.__setitem__("hook", h)
        mod.get_axon_ntff_profile_hook = lambda: _state["hook"]
        sys.modules["antenv.axon_hooks"] = mod
        antenv.axon_hooks = mod
        from trn_agent_boot.trn_boot import _ntff_profile_via_ctypes
        hook = _ntff_profile_via_ctypes("/opt/axon/libaxon_pjrt.so")
        if hook is not None:
            mod.set_axon_ntff_profile_hook(hook)
        return hook is not None
    except Exception:
        return False


def kernel(**inputs):
    global LAST_EXEC_TIME_NS, LAST_RESULTS
    from concourse.bass_utils import run_bass_kernel_spmd

    in_maps = prepare_shards(**inputs)
    nc = build_nc()

    trace = bool(int(os.environ.get("KERNEL_PROFILE", "0")))
    tmpdir = None
    if trace:
        trace = _ensure_profile_hook()
        if trace:
            tmpdir = os.environ.get("KERNEL_PROFILE_DIR") or None
    res = run_bass_kernel_spmd(nc, in_maps, list(range(N_CORES)), trace=trace,
                               tmpdir=tmpdir)
    LAST_EXEC_TIME_NS = res.exec_time_ns
    LAST_RESULTS = res

    pair_map = _core_pair_map()
    out = np.empty((B, S, H, DM), dtype=np.float32)
    for c in range(N_CORES):
        dev = np.asarray(res.results[c]["out"], dtype=np.float32)
        # [PPC, S//256, 128, 2*DM] -> [PPC, S, DM]
        dev = (dev.reshape(PPC, S // 256, 128, 2, DM)
               .transpose(0, 1, 3, 2, 4).reshape(PPC, S, DM))
        for s in range(PPC):
            b, h = pair_map[c][s]
            out[b, :, h, :] = dev[s]

    b_O = np.asarray(inputs["b_O"], dtype=np.float32)
    b_V = np.asarray(inputs["b_V"], dtype=np.float32)
    b_Q = np.asarray(inputs["b_Q"], dtype=np.float32)
    b_K = np.asarray(inputs["b_K"], dtype=np.float32)
    if np.any(b_Q) or np.any(b_K):
        raise NotImplementedError("nonzero b_Q/b_K not supported by this kernel")
    extra = b_O[None, :] / H  # [1, DM] broadcast over heads
    if np.any(b_V):
        extra = extra + np.einsum(
            "hd,hdm->hm", b_V, np.asarray(inputs["W_O"], dtype=np.float32))
    if np.any(extra):
        out = out + extra[None, None]
    return np.ascontiguousarray(out, dtype=np.float32)


# revision 54
# speedup vs baseline: 1.0024x; 1.0024x over previous
"""Trainium2 Bass kernel for per-head attention (TransformerLens-style).

Reference computation (per batch b, head h, with x = resid[b, :, h, :]):
    q = x @ W_Q[h] + b_Q[h];  k = x @ W_K[h] + b_K[h];  v = x @ W_V[h] + b_V[h]
    scores = q @ k.T / sqrt(DH), causal-masked, softmax over keys
    z = P @ v;  out[b, :, h, :] = z @ W_O[h] + b_O / H

Shapes: B=4, S=1024, H=12, DM=768, DH=64.  B*H = 48 independent attention
problems; 8 NeuronCores get 6 each (pure data parallel, no collectives).

v2 design notes (on top of the v1 couple/strip scheme -- see
kernel_v1_backup.py for the original docstring).  Measured best:
~164.6us (v1 baseline 164.6-166.5us).

  - LDWEIGHTS merging: bacc lowers every matmul to Ldweights+Matmult and a
    load costs ~107ns regardless of row count (cost scales with COLUMNS,
    streamed at 1.2 GHz independent of HAM).  _optimize_ldweights runs on
    the final SCHEDULED stream (the Tile scheduler reorders emission, so
    merge decisions must happen post-schedule): (1) row-merge two 64-row
    loads of vertical halves of one tile / col-merge two 64-col loads of
    one 128-col chunk into a single load (APs are mutated in place;
    same-semaphore ge-imm waits collapse to the max value, ISA wait-slot
    cap is 2); (2) cover-dedup repeats whose array region already holds
    identical data, forwarding their waits to the next PE instruction.
    Layouts built for (1): zT2[0:64]=even z^T strips / [64:128]=odd strips
    at the same columns (out-proj pairs), vT full-column transpose loads
    (the transpose stationary IS the data), host-side-duplicated [wv|wv]
    chunks (v-proj column-tile pair).  480 -> 334 loads.
  - l (softmax denominators): the z-psum row 64 (ones-column augmented v)
    is copied per-half into a [1, S] staging row, one gpsimd DMA scatters
    4 strips into partitions of an [8,128] lf tile (shape-mismatched
    SBUF-SBUF DMA: [1,512]->[4,128] is accepted and scatters by element
    order; DVE CANNOT write non-32-aligned partition bases), and ONE
    matmul against ident[0:8,0:8] transposes all 8 rows -> [128, 8]
    reciprocal (v1 used 8 ldweights+matmuls per pair for this).
  - PSUM 1-bank granules: "acc" ring bufs=4 rotates qk0lo,qk0hi,qk1lo,
    qk1hi,z0lo,z1lo,z0hi,z1hi per couple; z lo/hi split frees the lo bank
    mid-phase-B so the next couple's qk starts without waiting the z
    drain (v1's couple-boundary stalls ~1us).  vt/vtr/score share "scps"
    (bufs=2), out chunks + l + warmup share "ops" (bufs=2).  8 banks.
  - zT2 extraction: one strided-gather DVE copy per half (even strips ->
    top, odd -> bottom via the legal 32-aligned 0:64->64:128 partition
    shift); per-iteration on the last couple (lag-1 for the tail).
  - Cold start: ident loads first; ~40 dummy ident matmuls emitted before
    the qk loop warm the HAM clock-gate (4/8=1.2GHz default, releases to
    8/8=2.4GHz after ~3.4us sustained busy) during the DMA-bound ramp;
    couple-0 x loads split across sync(pair0)/gpsimd(pair1) queues and the
    bulk wqkv piece rides gpsimd so x chunks are never queued behind it.
  - Out-copies 5:3 DVE:Sc round-robin (ScalarE also carries all exps;
    measured Sc 69us / DVE 66us busy).

HAM/throttle reality measured across 11 HW runs: warm (K=8/8) fraction
rose 26% -> 33% as structure improved, so it IS schedule-coupled (not a
hard power cap), but the dominant pattern is: a ~1us PE gap at each
couple boundary re-throttles the clock-gate, and the ENTIRE next
projection phase (~20us) then runs at 1.2 GHz until phase B's density
releases it again.  Boundary gap cause: the next couple's qk matmuls
need acc-ring granules freed by the previous couple's z-hi extraction
(DVE) which competes with out-copies right at the boundary.  A pair-1
qk-stagger and moving the m=3 extraction to ScalarE were both tried and
MEASURED WORSE (+5us: ScalarE is not actually free at m=3, and the
stagger's stationary reloads cost more than the slack buys); the
granule-ring release order is what keeps the boundary gap at ~1us.
Engine floors: ScalarE
~70us (exps ~44us of it, per-ACTIVATE overhead (N+352)/1.2 ns), DVE
~67us, PE active ~130us at the throttled clock mix; MM concurrency
checks out (~40% of MM union at depth>=2 = the packable share; qk and z
are legitimately depth-1).

Dead ends measured on HW (do not retry blindly): fp8e4 DoubleRow matmuls
stream 1 output column/cycle (not the cost model's 0.5), so a hi+lo fp8
split costs 1.5x bf16; DoubleRow also cannot write a column-packed psum
dst at partition base 64 (ISA s3d3_mm_valid_dst_partition).  dma_start
rejects PSUM APs (no direct psum->DRAM store; all psum drains go through
Sc/DVE).  Scattering >70 warm-filler matmuls through the stream ADDS
~10us (they run mostly cold).  Strided-gather gpsimd DMAs (kkT-style
column compression) cost ~4x contiguous in DGE descriptor time and
clogged the gpsimd queue (+22us busy) -- keep gpsimd DMAs contiguous.
MM output psum dst must sit within ONE 2KB bank (512 f32) -- merged-exp
[128,1024] score tiles would need 4 scps banks (psum budget is exactly
8: acc 4 + scps 2 + ops 2).  ops bufs=1 fails walrus codegen (an MM ends
up with >2 sync waits: S3D3_MM wait-slot cap).  Gating the v-projection
behind qkT copies via acc-ring granules costs more (PE hole at the qk->v
transition) than the ~50 interleave-reloads it saves.  z col-tiling
across the couple's pairs (the remaining 2x on the z phase) is closed:
any l computation requires a second pt stream through the PE -- the only
free ride is the 65th stationary column, which is exactly what blocks
the packing.
"""

import os
import numpy as np
import ml_dtypes
from contextlib import ExitStack

B, S, H, DM, DH = 4, 1024, 12, 768, 64
N_CORES = 8
PAIRS = B * H
PPC = PAIRS // N_CORES      # pairs per core
CPC = PPC // 2              # couples per core

BF16 = ml_dtypes.bfloat16

LAST_EXEC_TIME_NS = None
LAST_RESULTS = None


def _core_pair_map():
    """(b, h) for each (core, slot).  Couple g = (head g//2, batch-half g%2);
    core c owns couples 3c..3c+2, slot s -> couple 3c + s//2, e = s%2."""
    m = []
    for c in range(N_CORES):
        row = []
        for s in range(PPC):
            g = 3 * c + s // 2
            h, bh, e = g // 2, g % 2, s % 2
            row.append((2 * bh + e, h))
        m.append(row)
    return m


def _strip_blocks(i, s_len):
    """128-aligned score blocks for strip i: start at the diagonal."""
    v = 128 * i
    if v < 512:
        return [(v, 512), (512, s_len)]
    return [(v, s_len)]


def build_nc(n_couples=CPC, s_len=S, dm=DM, dh=DH):
    import concourse.bacc as bacc
    import concourse.tile as tile
    import concourse.mybir as mybir

    f32 = mybir.dt.float32
    bf16 = mybir.dt.bfloat16
    KC = dm // 128
    NSQ = s_len // 128
    NM = NSQ // 2            # strip-pairs
    MMB = 512

    nc = bacc.Bacc("TRN2", target_bir_lowering=False, debug=False)

    WQK = KC * 2 * dh        # qk weight region columns
    WVW = KC * 128           # v weight region columns (wv duplicated)
    xt = nc.declare_dram_parameter("xt", [2 * n_couples, 128, KC * s_len], bf16, isOutput=False)
    wqkv = nc.declare_dram_parameter("wqkv", [n_couples, 128, WQK + WVW], bf16, isOutput=False)
    wo = nc.declare_dram_parameter("wo", [n_couples, 128, dm], bf16, isOutput=False)
    ident = nc.declare_dram_parameter("ident", [128, 128], bf16, isOutput=False)
    out = nc.declare_dram_parameter("out", [2 * n_couples, NM, 128, 2 * dm], bf16, isOutput=True)

    Exp = mybir.ActivationFunctionType.Exp
    WVO = WQK  # column offset of wv within wqkv

    with ExitStack() as ctx:
        tc = ctx.enter_context(tile.TileContext(nc))

        xt_pool = ctx.enter_context(tc.tile_pool(name="xt", bufs=2 * n_couples))
        wqkv_pool = ctx.enter_context(tc.tile_pool(name="wqkv", bufs=n_couples))
        wo_pool = ctx.enter_context(tc.tile_pool(name="wo", bufs=n_couples))
        const_pool = ctx.enter_context(tc.tile_pool(name="const", bufs=1))
        qkT_pool = ctx.enter_context(tc.tile_pool(name="qkT", bufs=3))
        qdup_pool = ctx.enter_context(tc.tile_pool(name="qdup", bufs=3))
        vT_pool = ctx.enter_context(tc.tile_pool(name="vT", bufs=2))
        vaug_pool = ctx.enter_context(tc.tile_pool(name="vaug", bufs=2))
        pstrip_pool = ctx.enter_context(tc.tile_pool(name="pstrip", bufs=12))
        zT2_pool = ctx.enter_context(tc.tile_pool(name="zT2", bufs=4))
        lf_pool = ctx.enter_context(tc.tile_pool(name="lf", bufs=4))
        stage_pool = ctx.enter_context(tc.tile_pool(name="stage", bufs=4))
        recip_pool = ctx.enter_context(tc.tile_pool(name="recip", bufs=4))
        osb_pool = ctx.enter_context(tc.tile_pool(name="osb", bufs=8))

        # PSUM (8 banks): acc = 4x 1-bank granules rotating
        # qk0lo,qk0hi,qk1lo,qk1hi,z0lo,z1lo,z0hi,z1hi per couple;
        # scps = vt granules + v-transposes + score blocks (2 banks);
        # ops = out-proj chunks + l columns (2 banks).
        acc_pool = ctx.enter_context(tc.tile_pool(name="acc", bufs=4, space="PSUM"))
        scps = ctx.enter_context(tc.tile_pool(name="scps", bufs=2, space="PSUM"))
        ops_pool = ctx.enter_context(tc.tile_pool(name="ops", bufs=2, space="PSUM"))

        # ---- loads are issued just-in-time, one couple ahead ----
        wqkv_sbs, wo_sbs, x_sbs = [], [], []
        kh = KC // 2

        def issue_couple_loads(g, fine):
            wqkv_sb = wqkv_pool.tile([128, WQK + WVW], bf16, name=f"wqkv_{g}", tag="wqkv")
            wo_sb = wo_pool.tile([128, dm], bf16, name=f"wo_{g}", tag="wo")
            if fine:
                # first couple: per-chunk pieces in consumption order so the
                # qk kc-loop never outruns the load stream
                nc.sync.dma_start(wqkv_sb[:, :2 * 2 * dh], wqkv[g, :, :2 * 2 * dh])
                xts = []
                for e in (0, 1):
                    p = 2 * g + e
                    xtile = xt_pool.tile([128, KC * s_len], bf16, name=f"x_{p}", tag="x")
                    xts.append(xtile)
                    x_sbs.append(xtile)
                # pair 0 on the sync queue, pair 1 on the gpsimd queue --
                # two DMA channels halve the cold-start load time
                XQ = {0: nc.sync, 1: nc.gpsimd}
                for e in (0, 1):
                    XQ[e].dma_start(xts[e][:, :512], xt[2 * g + e, :, :512])
                # bulk weight piece rides the gpsimd queue so it doesn't
                # delay pair-0's x chunks on the sync queue
                nc.gpsimd.dma_start(wqkv_sb[:, 2 * 2 * dh:], wqkv[g, :, 2 * 2 * dh:])
                for e in (0, 1):
                    XQ[e].dma_start(xts[e][:, 512:s_len], xt[2 * g + e, :, 512:s_len])
                for kc in range(1, KC):
                    for e in (0, 1):
                        XQ[e].dma_start(
                            xts[e][:, kc * s_len:(kc + 1) * s_len],
                            xt[2 * g + e, :, kc * s_len:(kc + 1) * s_len])
                nc.sync.dma_start(wo_sb[:], wo[g])
            else:
                nc.sync.dma_start(wqkv_sb[:], wqkv[g])
                for e in (0, 1):
                    p = 2 * g + e
                    xtile = xt_pool.tile([128, KC * s_len], bf16, name=f"x_{p}", tag="x")
                    nc.sync.dma_start(xtile[:, :kh * s_len], xt[p, :, :kh * s_len])
                    nc.sync.dma_start(xtile[:, kh * s_len:], xt[p, :, kh * s_len:])
                    x_sbs.append(xtile)
                nc.sync.dma_start(wo_sb[:], wo[g])
            wqkv_sbs.append(wqkv_sb)
            wo_sbs.append(wo_sb)

        # ident loads FIRST (tiny) so the HAM warm-up burst below can start
        # immediately; the real loads follow on the same queue
        ident_sb = const_pool.tile([128, 128], bf16, name="ident_sb")
        nc.sync.dma_start(ident_sb[:], ident[:, :])
        issue_couple_loads(0, fine=True)
        if n_couples > 1:
            issue_couple_loads(1, fine=False)

        # HAM warm-up/keep-warm fillers: the PE clock-gate defaults to 4/8
        # (1.2 GHz) and only releases to 8/8 after ~3.4us of sustained busy;
        # any ~us idle re-throttles.  The first couple is DMA-load-bound, so
        # dummy ident matmuls are sprinkled at priorities BETWEEN the real
        # work: the list scheduler only runs them when nothing else is ready,
        # keeping the PE array busy through load stalls.
        wu_ps = ops_pool.tile([128, 128], f32, name="warm_ps", tag="ops")

        def warm_fill(n):
            for _ in range(n):
                nc.tensor.matmul(wu_ps[:, 0:128], lhsT=ident_sb[:, :],
                                 rhs=ident_sb[:, :], start=True, stop=True,
                                 skip_group_check=True)

        # 60 matmuls x ~107ns bridges from DMA-queue boot (~3us) to the
        # first data-ready qk matmuls (~9.5us) with no idle window between
        warm_fill(60)

        # engine round-robin for out-copies (psum readers: DVE/ScalarE only);
        # 5:3 DVE:Sc because ScalarE also carries all the exps
        OUT_ENGS = [nc.vector, nc.scalar, nc.vector, nc.scalar,
                    nc.vector, nc.vector, nc.scalar, nc.vector]
        out_rr = [0]

        pending_out = []
        final_split = [False]

        def emit_one_pending():
            if pending_out:
                pending_out.pop(0)()

        def make_out(p, j, zT2_sb, recip_sb, wo_sb):
            """Out-projection for strip-couple (j, j+1): one merged 128-row
            ldweights of zT2 block m, row-packed dj matmuls."""
            m = j // 2
            e = p & 1

            def emit():
                o_sb = osb_pool.tile([128, 2 * dm], bf16, name=f"osb_{p}_{j}", tag="osb")
                for c0 in range(0, dm, MMB):
                    c1 = min(c0 + MMB, dm)
                    o_tiles = []
                    for dj in (0, 1):
                        o_ps = ops_pool.tile([128, 512], f32, name=f"ops_{p}_{j + dj}_{c0}", tag="ops")
                        nc.tensor.matmul(
                            o_ps[:, 0:c1 - c0],
                            lhsT=zT2_sb[64 * dj:64 * dj + dh,
                                        m * 128:(m + 1) * 128],
                            rhs=wo_sb[64 * dj:64 * dj + dh, c0:c1],
                            start=True, stop=True,
                        )
                        o_tiles.append(o_ps)
                    for dj in (0, 1):
                        dst = o_sb[:, dj * dm + c0:dj * dm + c1]
                        osrc = o_tiles[dj][:, 0:c1 - c0]
                        scal = recip_sb[:, e * 4 + (j + dj) % 4:e * 4 + (j + dj) % 4 + 1]
                        eng = OUT_ENGS[out_rr[0] % len(OUT_ENGS)]
                        out_rr[0] += 1
                        if eng is nc.scalar:
                            nc.scalar.mul(dst, osrc, scal)
                        else:
                            eng.tensor_scalar_mul(dst, osrc, scal)
                if final_split[0]:
                    # tail: halve store latency across two queues
                    nc.sync.dma_start(out[p, m][:, :dm], o_sb[:, :dm])
                    nc.gpsimd.dma_start(out[p, m][:, dm:], o_sb[:, dm:])
                else:
                    nc.sync.dma_start(out[p, m], o_sb[:])
            return emit

        for g in range(n_couples):
            if g + 2 < n_couples:
                issue_couple_loads(g + 2, fine=False)
            p0, p1 = 2 * g, 2 * g + 1
            x0, x1 = x_sbs[p0], x_sbs[p1]
            wqkv_sb = wqkv_sbs[g]
            wo_sb = wo_sbs[g]
            last = g == n_couples - 1

            # ---- qk^T projections into 1-bank granules, shared stationary ----
            qk_gr = {}
            for e in (0, 1):
                for hh in (0, 1):
                    qk_gr[e, hh] = acc_pool.tile(
                        [128, 512], f32, name=f"qkps_{2 * g + e}_{hh}", tag="acc")
            # kc-outer: the 4 granule matmuls of each kc share one stationary
            # (single ldweights after dedup); granule-ring slot release gives
            # the previous couple's z extraction enough slack at the boundary
            # (a 2-chunk pair-1 stagger was tried: 164.1us vs 161-163, no win)
            for kc in range(KC):
                for e, xtile in ((0, x0), (1, x1)):
                    for hh in (0, 1):
                        n0 = hh * 512
                        nc.tensor.matmul(
                            qk_gr[e, hh][:, 0:512],
                            lhsT=wqkv_sb[:, kc * 2 * dh:(kc + 1) * 2 * dh],
                            rhs=xtile[:, kc * s_len + n0:kc * s_len + n0 + 512],
                            start=(kc == 0), stop=(kc == KC - 1),
                            skip_group_check=(e == 1 or hh == 1),
                        )
            qkTs, qdups = [], []
            for e, p in ((0, p0), (1, p1)):
                qkT_sb = qkT_pool.tile([128, s_len], bf16, name=f"qkT_{p}", tag="qkT")
                # lo/hi on different engines so both copies run concurrently
                if e == 0:
                    nc.scalar.copy(qkT_sb[:, 0:512], qk_gr[e, 0][:, 0:512])
                    nc.vector.tensor_copy(qkT_sb[:, 512:1024], qk_gr[e, 1][:, 0:512])
                else:
                    nc.vector.tensor_copy(qkT_sb[:, 0:512], qk_gr[e, 0][:, 0:512])
                    nc.scalar.copy(qkT_sb[:, 512:1024], qk_gr[e, 1][:, 0:512])
                # partition swap: swap[0:64]=k^T, swap[64:128]=q^T -- two
                # contiguous [64,1024] DMAs (cheap descriptor count)
                swap_sb = qdup_pool.tile([128, s_len], bf16, name=f"swap_{p}", tag="qdup")
                nc.gpsimd.dma_start(swap_sb[0:dh, :], qkT_sb[dh:2 * dh, :])
                nc.gpsimd.dma_start(swap_sb[dh:2 * dh, :], qkT_sb[0:dh, :])
                qkTs.append(qkT_sb)
                qdups.append(swap_sb)
                emit_one_pending()

            # ---- v^T projections, column-packed via duplicated weights ----
            vt_gr = [scps.tile([128, 512], f32, name=f"vtps_{g}_{hh}", tag="scps")
                     for hh in (0, 1)]
            for kc in range(KC):
                for hh in (0, 1):
                    n0 = hh * 512
                    for e, xtile in ((0, x0), (1, x1)):
                        nc.tensor.matmul(
                            vt_gr[hh][64 * e:64 * e + dh, 0:512],
                            lhsT=wqkv_sb[:, WVO + kc * 128 + 64 * e:
                                         WVO + kc * 128 + 64 * e + dh],
                            rhs=xtile[:, kc * s_len + n0:kc * s_len + n0 + 512],
                            start=(kc == 0), stop=(kc == KC - 1),
                            skip_group_check=True,
                        )
            vT_sb = vT_pool.tile([128, s_len], bf16, name=f"vT_{g}", tag="vT")
            nc.vector.tensor_copy(vT_sb[:, 0:512], vt_gr[0][:, 0:512])
            nc.scalar.copy(vT_sb[:, 512:], vt_gr[1][:, 0:512])
            emit_one_pending()
            emit_one_pending()

            # bf16 transposes: one merged 128-row transpose-load per t-block
            vtrs = [scps.tile([128, NSQ * dh], bf16, name=f"vtr_{2 * g + e}", tag="scps")
                    for e in (0, 1)]
            for t in range(NSQ):
                for e in (0, 1):
                    nc.tensor.transpose(
                        vtrs[e][:, t * dh:(t + 1) * dh],
                        vT_sb[64 * e:64 * e + dh, t * 128:(t + 1) * 128],
                        ident_sb[64 * e:64 * e + dh, 64 * e:64 * e + dh],
                    )
            vaugs = []
            for e, p in ((0, p0), (1, p1)):
                vaug_sb = vaug_pool.tile([128, NSQ * (dh + 1)], bf16, name=f"vaug_{p}", tag="vaug")
                if g == 0:
                    # ones columns persist across pool reuse; set once
                    nc.gpsimd.memset(vaug_sb[:], 1.0)
                nc.vector.tensor_copy(
                    vaug_sb[:].rearrange("p (n d) -> p n d", d=dh + 1)[:, :, 0:dh],
                    vtrs[e][:].rearrange("p (n d) -> p n d", d=dh),
                )
                vaugs.append(vaug_sb)

            # ---- phase B: pairs interleaved per strip-pair ----
            z_gr, zT2_sbs = {}, {}
            lf_sbs = {}
            recip_sbs = {}
            for e, p in ((0, p0), (1, p1)):
                zT2_sbs[e] = zT2_pool.tile([128, NM * 128], bf16, name=f"zT2_{p}", tag="zT2")
            # z granules allocated in release-friendly order
            for e, p in ((0, p0), (1, p1)):
                z_gr[e, 0] = acc_pool.tile([128, 512], f32, name=f"zps_{p}_lo", tag="acc")
            for e, p in ((0, p0), (1, p1)):
                z_gr[e, 1] = acc_pool.tile([128, 512], f32, name=f"zps_{p}_hi", tag="acc")
            for hh in (0, 1):
                lf_sbs[hh] = lf_pool.tile([8, 128], bf16, name=f"lf_{g}_{hh}", tag="lf")
                recip_sbs[hh] = recip_pool.tile([128, 8], f32, name=f"recip_{g}_{hh}", tag="recip")
            stage_sbs = {e: stage_pool.tile([1, s_len], bf16, name=f"lstage_{2 * g + e}", tag="stage")
                         for e in (0, 1)}

            for m in range(NM):
                i0 = 2 * m
                blocks0 = _strip_blocks(i0, s_len)
                blocks1 = _strip_blocks(i0 + 1, s_len)
                nblk = max(len(blocks0), len(blocks1))
                sc_tiles = {}
                # row-packed score matmuls for BOTH pairs (k^T strips as
                # stationaries in opposite partition halves)
                for e, p in ((0, p0), (1, p1)):
                    qkT_sb, swap_sb = qkTs[e], qdups[e]
                    for bi in range(nblk):
                        for di, i, blocks in ((0, i0, blocks0), (1, i0 + 1, blocks1)):
                            bj = bi - (nblk - len(blocks))
                            if bj < 0:
                                continue
                            a, b = blocks[bj]
                            sc_ps = scps.tile([128, 512], f32, name=f"sc_{p}_{i}_{a}", tag="scps")
                            if di == 0:
                                lhsT = swap_sb[0:dh, i * 128:(i + 1) * 128]
                                rhs = qkT_sb[0:dh, a:b]
                            else:
                                lhsT = qkT_sb[dh:2 * dh, i * 128:(i + 1) * 128]
                                rhs = swap_sb[dh:2 * dh, a:b]
                            nc.tensor.matmul(
                                sc_ps[:, 0:b - a], lhsT=lhsT, rhs=rhs,
                                start=True, stop=True,
                            )
                            sc_tiles[(e, i, a)] = sc_ps

                # PE gap fillers: deferred out-couples run here
                emit_one_pending()
                emit_one_pending()

                # exp (ScalarE), diag mask (GpSimd), z matmuls, extraction
                for e, p in ((0, p0), (1, p1)):
                    vaug_sb = vaugs[e]
                    zT2_sb = zT2_sbs[e]
                    for di, i, blocks in ((0, i0, blocks0), (1, i0 + 1, blocks1)):
                        # all exps of the strip first, then both z matmuls
                        # back-to-back so the vaug stationary loads once
                        pts = []
                        for (a, b) in blocks:
                            sc_ps = sc_tiles[(e, i, a)]
                            pt_sb = pstrip_pool.tile([128, 512], bf16, name=f"pt_{p}_{i}_{a}", tag="pstrip")
                            nc.scalar.activation(pt_sb[:, 0:b - a], sc_ps[:, 0:b - a], Exp)
                            if a == 128 * i:  # leading block holds the diag triangle
                                dst = pt_sb[:, 0:128]
                                nc.gpsimd.affine_select(
                                    out=dst, in_=dst,
                                    compare_op=mybir.AluOpType.is_ge,
                                    fill=0.0, base=0,
                                    pattern=[[1, 128]], channel_multiplier=-1,
                                )
                            pts.append(pt_sb)
                        for (a, b), pt_sb in zip(blocks, pts):
                            # z dst granule(s): blocks never straddle col 512
                            gr = z_gr[e, 0] if b <= 512 else z_gr[e, 1]
                            goff = 0 if b <= 512 else 512
                            nc.tensor.matmul(
                                gr[0:dh + 1, a - goff:b - goff],
                                lhsT=vaug_sb[:, i * (dh + 1):(i + 1) * (dh + 1)],
                                rhs=pt_sb[:, 0:b - a],
                                start=(i == 0), stop=(i == (b - 1) // 128),
                                skip_group_check=True,
                            )

                    # eager extraction: even strips -> zT2 top half, odd ->
                    # bottom (DVE psum partitions 0:64 -> sbuf 64:128 is a
                    # legal 32-aligned shift).  Non-last couples defer to one
                    # strided-gather copy per half (half the instruction
                    # overhead); the last couple stays per-iteration (lag 1).
                    hh = m // 2
                    gr = z_gr[e, 0] if m < 2 else z_gr[e, 1]
                    c0 = 256 * m - (0 if m < 2 else 512)
                    if last:
                        nc.vector.tensor_copy(
                            zT2_sb[0:dh, m * 128:(m + 1) * 128], gr[0:dh, c0:c0 + 128])
                        nc.vector.tensor_copy(
                            zT2_sb[dh:2 * dh, m * 128:(m + 1) * 128], gr[0:dh, c0 + 128:c0 + 256])
                        nc.vector.tensor_copy(
                            stage_sbs[e][0:1, 256 * m:256 * m + 256], gr[dh:dh + 1, c0:c0 + 256])
                    elif m in (1, 3):
                        hv = gr[:, 0:512].rearrange("p (t o b) -> p t o b", o=2, b=128)
                        zv = zT2_sb[:, (m - 1) * 128:(m + 1) * 128].rearrange(
                            "p (t b) -> p t b", b=128)
                        nc.vector.tensor_copy(zv[0:dh], hv[0:dh, :, 0, :])
                        nc.vector.tensor_copy(zv[dh:2 * dh], hv[0:dh, :, 1, :])
                        nc.vector.tensor_copy(
                            stage_sbs[e][0:1, 512 * hh:512 * (hh + 1)],
                            gr[dh:dh + 1, 0:512])

                # l transpose: one 8-row matmul per couple-half gives both
                # pairs' denominator columns; last couple runs it per
                # iteration (lag-1) to shrink the final flush
                if m in (1, 3) or last:
                    hh = m // 2
                    lf_sb, recip_sb = lf_sbs[hh], recip_sbs[hh]
                    for e in (0, 1):
                        if last:
                            r0 = e * 4 + (2 * m) % 4
                            nc.gpsimd.dma_start(
                                lf_sb[r0:r0 + 2, 0:128],
                                stage_sbs[e][0:1, 256 * m:256 * (m + 1)])
                        else:
                            nc.gpsimd.dma_start(
                                lf_sb[e * 4:e * 4 + 4, 0:128],
                                stage_sbs[e][0:1, 512 * hh:512 * (hh + 1)])
                    l_ps = ops_pool.tile([128, 8], f32, name=f"lps_{g}_{m}", tag="ops")
                    nc.tensor.matmul(
                        l_ps[:, 0:8], lhsT=lf_sb[0:8, 0:128],
                        rhs=ident_sb[0:8, 0:8], start=True, stop=True,
                    )
                    nc.vector.reciprocal(recip_sb[:, 0:8], l_ps[:, 0:8])
                    if last:
                        for e, p in ((0, p0), (1, p1)):
                            pending_out.append(make_out(p, 2 * m, zT2_sbs[e], recip_sb, wo_sb))
                    else:
                        for e, p in ((0, p0), (1, p1)):
                            pending_out.append(make_out(p, 2 * m - 2, zT2_sbs[e], recip_sb, wo_sb))
                            pending_out.append(make_out(p, 2 * m, zT2_sbs[e], recip_sb, wo_sb))

        final_split[0] = True
        while pending_out:
            emit_one_pending()

    nc.finalize()
    _optimize_ldweights(nc, mybir)
    return nc


def _optimize_ldweights(nc, mybir):
    """Merge and dedup PE Ldweights on the final scheduled stream.

    bacc lowers every matmul to an Ldweights+Matmult pair, walrus runs with
    --enable-ldw-opt=false, and ldweights streams at ~1.2 GHz with cost
    proportional to its COLUMN count only -- a 128-row load costs the same
    as a 64-row one.  Two rewrites:

    1. pair-merge: a row-packed pair loads vertical halves of the same tile
       at the same columns (kkT/zT2/vT layouts are built for this), and the
       duplicated-wv layout loads horizontal halves of one 128-col chunk.
       The earlier load's AP is widened to cover both and the later load is
       deleted (its waits join the merged load: both halves' producers must
       be done before the single load streams).
    2. cover-dedup: a load is dropped when the array region it would fill
       already holds identical data (repeat stationaries across moving
       blocks); its waits move onto the next PE instruction.

    Merging is restricted to a tensor-name safe-list whose producers never
    depend on PE progress between the pair (no wait-cycle risk)."""
    pe = mybir.EngineType.PE
    MERGE_OK = {"kkT", "zT2", "vT", "wqkv"}
    removed = 0

    def parse(inst):
        ap = inst.ins[0]
        a = [list(x) for x in ap.ap]
        if len(a) != 2 or a[1][0] != 1:
            return None
        (pstride, np_), (_, ncols) = a
        if pstride <= 0:
            return None
        pbase, coff = divmod(ap.offset, pstride)
        tp = inst.tile_position or (0, 0)
        return dict(mem=ap.memref, dt=repr(ap.dtype), np=np_, ncols=ncols,
                    pstride=pstride, pbase=pbase, coff=coff, tp=tp,
                    it=repr(inst.is_transpose), pm=repr(inst.perf_mode))

    def region(d):
        return (d["tp"][0], d["tp"][0] + d["np"], d["tp"][1], d["tp"][1] + d["ncols"])

    def overlaps(a, b):
        return a[0] < b[1] and b[0] < a[1] and a[2] < b[3] and b[2] < a[3]

    def covers(k, c):
        if (k["mem"] != c["mem"] or k["dt"] != c["dt"] or k["it"] != c["it"]
                or k["pm"] != c["pm"]):
            return False
        if k["tp"][0] - k["pbase"] != c["tp"][0] - c["pbase"]:
            return False
        if k["tp"][1] - k["coff"] != c["tp"][1] - c["coff"]:
            return False
        rk, rc = region(k), region(c)
        return rk[0] <= rc[0] and rc[1] <= rk[1] and rk[2] <= rc[2] and rc[3] <= rk[3]

    def same_meta(x, y):
        return (x["mem"] == y["mem"] and x["dt"] == y["dt"] and x["it"] == y["it"]
                and x["pm"] == y["pm"] and x["pstride"] == y["pstride"])

    def pair_kind(x, y):
        """row: vertical halves (top at (0,c), bottom at (64,c), same cols);
        col: horizontal halves of one 128-col chunk at (0,0)/(0,64)."""
        if not same_meta(x, y) or x["mem"].split("_")[0] not in MERGE_OK:
            return None
        if (x["np"] == 64 and y["np"] == 64 and x["ncols"] == y["ncols"]
                and x["coff"] == y["coff"] and x["tp"][1] == y["tp"][1]
                and {(x["pbase"], x["tp"][0]), (y["pbase"], y["tp"][0])}
                == {(0, 0), (64, 64)}):
            return "row"
        if (x["np"] == 128 and y["np"] == 128 and x["ncols"] == 64 and y["ncols"] == 64
                and x["pbase"] == 0 and y["pbase"] == 0
                and {(x["coff"] - min(x["coff"], y["coff"]), x["tp"][1]),
                     (y["coff"] - min(x["coff"], y["coff"]), y["tp"][1])}
                == {(0, 0), (64, 64)}):
            return "col"
        return None

    def apply_merge(inst, x, y, kind):
        ap = inst.ins[0]
        if kind == "row":
            ap.offset = x["coff"]
            ap.ap = [[x["pstride"], 128], [1, x["ncols"]]]
            inst.tile_position = (0, x["tp"][1])
            inst.tile_size = (128, x["ncols"])
            return dict(x, np=128, pbase=0, coff=x["coff"], tp=(0, x["tp"][1]))
        else:
            c0 = min(x["coff"], y["coff"])
            ap.offset = c0
            ap.ap = [[x["pstride"], 128], [1, 128]]
            inst.tile_position = (0, 0)
            inst.tile_size = (128, 128)
            return dict(x, ncols=128, coff=c0, tp=(0, 0))

    MAX_WAITS = 2

    def dedup_waits(waits):
        """Collapse same-semaphore ge-imm waits to the max value."""
        out = []
        best = {}
        for w in waits:
            if getattr(w, "wait_mode", None) == "sem-ge-imm" and w.wait_reg is None:
                key = (w.sync_type, w.id)
                cur = best.get(key)
                if cur is None:
                    best[key] = w
                    out.append(w)
                elif w.wait_value > cur.wait_value:
                    out[out.index(cur)] = w
                    best[key] = w
            else:
                out.append(w)
        return out

    def try_merge_sync(dst_inst, src_inst):
        """Combine src's waits/updates into dst; False if over the ISA cap."""
        ssi = src_inst.sync_info
        dsi = dst_inst.sync_info
        waits = dedup_waits(
            (list(dsi.on_wait) if dsi else []) + (list(ssi.on_wait) if ssi else []))
        upds = (list(dsi.on_update) if dsi else []) + (list(ssi.on_update) if ssi else [])
        if len(waits) > MAX_WAITS:
            return False
        if dsi is None:
            dst_inst.sync_info = mybir.SyncInfo(on_wait=waits, on_update=upds)
        else:
            dsi.on_wait = waits
            dsi.on_update = upds
        return True

    ES = getattr(mybir, "InstEventSemaphore", ())

    for fn in nc.m.functions:
        for blk in fn.blocks:
            insts = blk.instructions
            n = len(insts)
            pe_idx = [i for i in range(n)
                      if getattr(insts[i], "engine", None) == pe]
            drop = set()

            # ---- pass 1: pair-merge on the PE stream ----
            k = 0
            while k < len(pe_idx):
                i = pe_idx[k]
                inst = insts[i]
                if i in drop or not isinstance(inst, mybir.InstLdweights):
                    k += 1
                    continue
                x = parse(inst)
                if x is None or x["mem"].split("_")[0] not in MERGE_OK:
                    k += 1
                    continue
                # scan forward for the partner: MMs/event-sems may intervene
                j = k + 1
                partner = None
                mid_mms = []
                while j < len(pe_idx):
                    inst2 = insts[pe_idx[j]]
                    if isinstance(inst2, mybir.InstLdweights):
                        y = parse(inst2)
                        if y is not None and pair_kind(x, y):
                            partner = (pe_idx[j], inst2, y)
                        break
                    if isinstance(inst2, mybir.InstMatmult):
                        mid_mms.append(inst2)
                        j += 1
                        continue
                    if ES and isinstance(inst2, ES):
                        j += 1
                        continue
                    break
                if partner is None:
                    k += 1
                    continue
                jj, y_inst, y = partner
                # the partner's region gets loaded earlier than before: no
                # intervening matmul may be streaming through it
                yreg = region(y) if pair_kind(x, y) == "row" else (
                    0, 128, y["tp"][1], y["tp"][1] + y["ncols"])
                bad = False
                for mm in mid_mms:
                    tp = mm.tile_position or (0, 0)
                    ts = mm.tile_size or (128, 128)
                    if overlaps((tp[0], tp[0] + ts[0], tp[1], tp[1] + ts[1]), yreg):
                        bad = True
                        break
                if bad or not try_merge_sync(inst, y_inst):
                    k += 1
                    continue
                apply_merge(inst, x, y, pair_kind(x, y))
                drop.add(jj)
                removed += 1
                k += 1

            # ---- pass 2: cover-dedup; waits forward to the next PE inst ----
            live = [ii for ii in pe_idx if ii not in drop]
            kept = []
            for pos, ii in enumerate(live):
                inst = insts[ii]
                if isinstance(inst, mybir.InstLdweights):
                    d = parse(inst)
                    si = inst.sync_info
                    no_upd = si is None or not si.on_update
                    if d is not None and no_upd and any(covers(kk, d) for kk in kept):
                        ok = True
                        if si is not None and si.on_wait:
                            ok = (pos + 1 < len(live)
                                  and try_merge_sync(insts[live[pos + 1]], inst))
                        if ok:
                            removed += 1
                            drop.add(ii)
                            continue
                    if d is not None:
                        kept = [kk for kk in kept if not overlaps(region(kk), region(d))]
                        kept.append(d)
                    else:
                        kept = []
                elif isinstance(inst, mybir.InstMatmult):
                    pass
                elif ES and isinstance(inst, ES):
                    pass
                else:
                    kept = []

            if drop:
                keep = [insts[i] for i in range(n) if i not in drop]
                del blk.instructions[:]
                for inst in keep:
                    blk.instructions.append(inst)
    return removed


def prepare_shards(normalized_resid_pre, W_Q, b_Q, W_K, b_K, W_V, b_V, W_O, b_O):
    """Host-side layout: returns in_maps for the 8 cores."""
    x = np.asarray(normalized_resid_pre, dtype=np.float32)
    scale = 1.0 / np.sqrt(DH)
    KC = DM // 128

    pair_map = _core_pair_map()

    # x^T per (core, slot), partition-major: [128, KC*S]
    xt_f = x.transpose(0, 2, 3, 1)  # [B, H, DM, S]
    # W_Q pre-scaled by 1/sqrt(DH) so scores come out pre-scaled
    wqk_h = np.concatenate([np.asarray(W_Q) * scale, np.asarray(W_K)], axis=-1)
    wv_h = np.asarray(W_V)  # [H, DM, DH]
    wo_h = np.asarray(W_O)  # [H, DH, DM]

    ident = np.eye(128).astype(BF16)

    in_maps = []
    for c in range(N_CORES):
        xts, wqkvs, wos = [], [], []
        for s in range(PPC):
            b, h = pair_map[c][s]
            xts.append(
                xt_f[b, h].reshape(KC, 128, S).transpose(1, 0, 2).reshape(128, KC * S))
            if s % 2 == 0:
                wv_c = wv_h[h].reshape(KC, 128, DH).transpose(1, 0, 2)  # [128, KC, DH]
                wv_dup = np.concatenate([wv_c, wv_c], axis=2)  # [128, KC, 128]
                wqkvs.append(np.concatenate(
                    [wqk_h[h].reshape(KC, 128, 2 * DH).transpose(1, 0, 2).reshape(128, KC * 2 * DH),
                     wv_dup.reshape(128, KC * 128)],
                    axis=1))
                wos.append(np.concatenate([wo_h[h], wo_h[h]], axis=0))  # [128, DM]
        in_maps.append({
            "xt": np.ascontiguousarray(np.stack(xts)).astype(BF16),
            "wqkv": np.ascontiguousarray(np.stack(wqkvs)).astype(BF16),
            "wo": np.ascontiguousarray(np.stack(wos)).astype(BF16),
            "ident": ident,
        })
    return in_maps


def _ensure_profile_hook():
    """The agent image lacks ``antenv.axon_hooks``; shim it and install the
    ctypes NTFF hook from trn_boot so trace=True works under axon."""
    import importlib
    import sys
    import types
    try:
        importlib.import_module("antenv.axon_hooks")
        return True
    except ImportError:
        pass
    try:
        import antenv
        mod = types.ModuleType("antenv.axon_hooks")
        _state = {"hook": None}
        mod.set_axon_ntff_profile_hook = lambda h: _state.__setitem__("hook", h)
        mod.get_axon_ntff_profile_hook = lambda: _state["hook"]
        sys.modules["antenv.axon_hooks"] = mod
        antenv.axon_hooks = mod
        from trn_agent_boot.trn_boot import _ntff_profile_via_ctypes
        hook = _ntff_profile_via_ctypes("/opt/axon/libaxon_pjrt.so")
        if hook is not None:
            mod.set_axon_ntff_profile_hook(hook)
        return hook is not None
    except Exception:
        return False


def kernel(**inputs):
    global LAST_EXEC_TIME_NS, LAST_RESULTS
    from concourse.bass_utils import run_bass_kernel_spmd

    in_maps = prepare_shards(**inputs)
    nc = build_nc()

    trace = bool(int(os.environ.get("KERNEL_PROFILE", "0")))
    tmpdir = None
    if trace:
        trace = _ensure_profile_hook()
        if trace:
            tmpdir = os.environ.get("KERNEL_PROFILE_DIR") or None
    res = run_bass_kernel_spmd(nc, in_maps, list(range(N_CORES)), trace=trace,
                               tmpdir=tmpdir)
    LAST_EXEC_TIME_NS = res.exec_time_ns
    LAST_RESULTS = res

    pair_map = _core_pair_map()
    out = np.empty((B, S, H, DM), dtype=np.float32)
    for c in range(N_CORES):
        dev = np.asarray(res.results[c]["out"], dtype=np.float32)
        # [PPC, S//256, 128, 2*DM] -> [PPC, S, DM]
        dev = (dev.reshape(PPC, S // 256, 128, 2, DM)
               .transpose(0, 1, 3, 2, 4).reshape(PPC, S, DM))
        for s in range(PPC):
            b, h = pair_map[c][s]
            out[b, :, h, :] = dev[s]

    b_O = np.asarray(inputs["b_O"], dtype=np.float32)
    b_V = np.asarray(inputs["b_V"], dtype=np.float32)
    b_Q = np.asarray(inputs["b_Q"], dtype=np.float32)
    b_K = np.asarray(inputs["b_K"], dtype=np.float32)
    if np.any(b_Q) or np.any(b_K):
        raise NotImplementedError("nonzero b_Q/b_K not supported by this kernel")
    extra = b_O[None, :] / H  # [1, DM] broadcast over heads
    if np.any(b_V):
        extra = extra + np.einsum(
            "hd,hdm->hm", b_V, np.asarray(inputs["W_O"], dtype=np.float32))
    if np.any(extra):
        out = out + extra[None, None]
    return np.ascontiguousarray(out, dtype=np.float32)


# revision 55
# speedup vs baseline: 1.0087x; 1.0062x over previous
"""Trainium2 Bass kernel for per-head attention (TransformerLens-style).

Reference computation (per batch b, head h, with x = resid[b, :, h, :]):
    q = x @ W_Q[h] + b_Q[h];  k = x @ W_K[h] + b_K[h];  v = x @ W_V[h] + b_V[h]
    scores = q @ k.T / sqrt(DH), causal-masked, softmax over keys
    z = P @ v;  out[b, :, h, :] = z @ W_O[h] + b_O / H

Shapes: B=4, S=1024, H=12, DM=768, DH=64.  B*H = 48 independent attention
problems; 8 NeuronCores get 6 each (pure data parallel, no collectives).

v2 design notes (on top of the v1 couple/strip scheme -- see
kernel_v1_backup.py for the original docstring).  Measured best:
~164.6us (v1 baseline 164.6-166.5us).

  - LDWEIGHTS merging: bacc lowers every matmul to Ldweights+Matmult and a
    load costs ~107ns regardless of row count (cost scales with COLUMNS,
    streamed at 1.2 GHz independent of HAM).  _optimize_ldweights runs on
    the final SCHEDULED stream (the Tile scheduler reorders emission, so
    merge decisions must happen post-schedule): (1) row-merge two 64-row
    loads of vertical halves of one tile / col-merge two 64-col loads of
    one 128-col chunk into a single load (APs are mutated in place;
    same-semaphore ge-imm waits collapse to the max value, ISA wait-slot
    cap is 2); (2) cover-dedup repeats whose array region already holds
    identical data, forwarding their waits to the next PE instruction.
    Layouts built for (1): zT2[0:64]=even z^T strips / [64:128]=odd strips
    at the same columns (out-proj pairs), vT full-column transpose loads
    (the transpose stationary IS the data), host-side-duplicated [wv|wv]
    chunks (v-proj column-tile pair).  480 -> 334 loads.
  - l (softmax denominators): the z-psum row 64 (ones-column augmented v)
    is copied per-half into a [1, S] staging row, one gpsimd DMA scatters
    4 strips into partitions of an [8,128] lf tile (shape-mismatched
    SBUF-SBUF DMA: [1,512]->[4,128] is accepted and scatters by element
    order; DVE CANNOT write non-32-aligned partition bases), and ONE
    matmul against ident[0:8,0:8] transposes all 8 rows -> [128, 8]
    reciprocal (v1 used 8 ldweights+matmuls per pair for this).
  - PSUM 1-bank granules: "acc" ring bufs=4 rotates qk0lo,qk0hi,qk1lo,
    qk1hi,z0lo,z1lo,z0hi,z1hi per couple; z lo/hi split frees the lo bank
    mid-phase-B so the next couple's qk starts without waiting the z
    drain (v1's couple-boundary stalls ~1us).  vt/vtr/score share "scps"
    (bufs=2), out chunks + l + warmup share "ops" (bufs=2).  8 banks.
  - zT2 extraction: one strided-gather DVE copy per half (even strips ->
    top, odd -> bottom via the legal 32-aligned 0:64->64:128 partition
    shift); per-iteration on the last couple (lag-1 for the tail).
  - Cold start: ident loads first; ~40 dummy ident matmuls emitted before
    the qk loop warm the HAM clock-gate (4/8=1.2GHz default, releases to
    8/8=2.4GHz after ~3.4us sustained busy) during the DMA-bound ramp;
    couple-0 x loads split across sync(pair0)/gpsimd(pair1) queues and the
    bulk wqkv piece rides gpsimd so x chunks are never queued behind it.
  - Out-copies 5:3 DVE:Sc round-robin (ScalarE also carries all exps;
    measured Sc 69us / DVE 66us busy).

HAM/throttle reality measured across 11 HW runs: warm (K=8/8) fraction
rose 26% -> 33% as structure improved, so it IS schedule-coupled (not a
hard power cap), but the dominant pattern is: a ~1us PE gap at each
couple boundary re-throttles the clock-gate, and the ENTIRE next
projection phase (~20us) then runs at 1.2 GHz until phase B's density
releases it again.  Boundary gap cause: the next couple's qk matmuls
need acc-ring granules freed by the previous couple's z-hi extraction
(DVE) which competes with out-copies right at the boundary.  A pair-1
qk-stagger and moving the m=3 extraction to ScalarE were both tried and
MEASURED WORSE (+5us: ScalarE is not actually free at m=3, and the
stagger's stationary reloads cost more than the slack buys); the
granule-ring release order is what keeps the boundary gap at ~1us.
Engine floors: ScalarE
~70us (exps ~44us of it, per-ACTIVATE overhead (N+352)/1.2 ns), DVE
~67us, PE active ~130us at the throttled clock mix; MM concurrency
checks out (~40% of MM union at depth>=2 = the packable share; qk and z
are legitimately depth-1).  Across all runs the warm windows are ALWAYS
exactly 4-5 x 3413ns and cold stretches 7-10 windows -- an apparent
~30-35% warm-duty ceiling (effective ~1.6 GHz).  At that clock, 204k
MM cycles + ~36us ldweights + ~10us start/tail reproduces the measured
161-164us exactly: this schedule sits at the platform's effective
roofline for its instruction mix.

Dead ends measured on HW (do not retry blindly): fp8e4 DoubleRow matmuls
stream 1 output column/cycle (not the cost model's 0.5), so a hi+lo fp8
split costs 1.5x bf16; DoubleRow also cannot write a column-packed psum
dst at partition base 64 (ISA s3d3_mm_valid_dst_partition).  dma_start
rejects PSUM APs (no direct psum->DRAM store; all psum drains go through
Sc/DVE).  Scattering >70 warm-filler matmuls through the stream ADDS
~10us (they run mostly cold).  Strided-gather gpsimd DMAs (kkT-style
column compression) cost ~4x contiguous in DGE descriptor time and
clogged the gpsimd queue (+22us busy) -- keep gpsimd DMAs contiguous.
MM output psum dst must sit within ONE 2KB bank (512 f32) -- merged-exp
[128,1024] score tiles would need 4 scps banks (psum budget is exactly
8: acc 4 + scps 2 + ops 2).  ops bufs=1 fails walrus codegen (an MM ends
up with >2 sync waits: S3D3_MM wait-slot cap).  Gating the v-projection
behind qkT copies via acc-ring granules costs more (PE hole at the qk->v
transition) than the ~50 interleave-reloads it saves.  z col-tiling
across the couple's pairs (the remaining 2x on the z phase) is closed:
any l computation requires a second pt stream through the PE -- the only
free ride is the 65th stationary column, which is exactly what blocks
the packing.
"""

import os
import numpy as np
import ml_dtypes
from contextlib import ExitStack

B, S, H, DM, DH = 4, 1024, 12, 768, 64
N_CORES = 8
PAIRS = B * H
PPC = PAIRS // N_CORES      # pairs per core
CPC = PPC // 2              # couples per core

BF16 = ml_dtypes.bfloat16

LAST_EXEC_TIME_NS = None
LAST_RESULTS = None


def _core_pair_map():
    """(b, h) for each (core, slot).  Couple g = (head g//2, batch-half g%2);
    core c owns couples 3c..3c+2, slot s -> couple 3c + s//2, e = s%2."""
    m = []
    for c in range(N_CORES):
        row = []
        for s in range(PPC):
            g = 3 * c + s // 2
            h, bh, e = g // 2, g % 2, s % 2
            row.append((2 * bh + e, h))
        m.append(row)
    return m


def _strip_blocks(i, s_len):
    """128-aligned score blocks for strip i: start at the diagonal."""
    v = 128 * i
    if v < 512:
        return [(v, 512), (512, s_len)]
    return [(v, s_len)]


def build_nc(n_couples=CPC, s_len=S, dm=DM, dh=DH):
    import concourse.bacc as bacc
    import concourse.tile as tile
    import concourse.mybir as mybir

    f32 = mybir.dt.float32
    bf16 = mybir.dt.bfloat16
    KC = dm // 128
    NSQ = s_len // 128
    NM = NSQ // 2            # strip-pairs
    MMB = 512

    nc = bacc.Bacc("TRN2", target_bir_lowering=False, debug=False)

    WQK = KC * 2 * dh        # qk weight region columns
    WVW = KC * 128           # v weight region columns (wv duplicated)
    xt = nc.declare_dram_parameter("xt", [2 * n_couples, 128, KC * s_len], bf16, isOutput=False)
    wqkv = nc.declare_dram_parameter("wqkv", [n_couples, 128, WQK + WVW], bf16, isOutput=False)
    wo = nc.declare_dram_parameter("wo", [n_couples, 128, dm], bf16, isOutput=False)
    ident = nc.declare_dram_parameter("ident", [128, 128], bf16, isOutput=False)
    out = nc.declare_dram_parameter("out", [2 * n_couples, NM, 128, 2 * dm], bf16, isOutput=True)

    Exp = mybir.ActivationFunctionType.Exp
    WVO = WQK  # column offset of wv within wqkv

    with ExitStack() as ctx:
        tc = ctx.enter_context(tile.TileContext(nc))

        xt_pool = ctx.enter_context(tc.tile_pool(name="xt", bufs=2 * n_couples))
        wqkv_pool = ctx.enter_context(tc.tile_pool(name="wqkv", bufs=n_couples))
        wo_pool = ctx.enter_context(tc.tile_pool(name="wo", bufs=n_couples))
        const_pool = ctx.enter_context(tc.tile_pool(name="const", bufs=1))
        qkT_pool = ctx.enter_context(tc.tile_pool(name="qkT", bufs=3))
        qdup_pool = ctx.enter_context(tc.tile_pool(name="qdup", bufs=3))
        vT_pool = ctx.enter_context(tc.tile_pool(name="vT", bufs=2))
        vaug_pool = ctx.enter_context(tc.tile_pool(name="vaug", bufs=2))
        pstrip_pool = ctx.enter_context(tc.tile_pool(name="pstrip", bufs=12))
        zT2_pool = ctx.enter_context(tc.tile_pool(name="zT2", bufs=4))
        lf_pool = ctx.enter_context(tc.tile_pool(name="lf", bufs=4))
        stage_pool = ctx.enter_context(tc.tile_pool(name="stage", bufs=4))
        recip_pool = ctx.enter_context(tc.tile_pool(name="recip", bufs=4))
        osb_pool = ctx.enter_context(tc.tile_pool(name="osb", bufs=8))

        # PSUM (8 banks): acc = 4x 1-bank granules rotating
        # qk0lo,qk0hi,qk1lo,qk1hi,z0lo,z1lo,z0hi,z1hi per couple;
        # scps = vt granules + v-transposes + score blocks (2 banks);
        # ops = out-proj chunks + l columns (2 banks).
        acc_pool = ctx.enter_context(tc.tile_pool(name="acc", bufs=4, space="PSUM"))
        scps = ctx.enter_context(tc.tile_pool(name="scps", bufs=2, space="PSUM"))
        ops_pool = ctx.enter_context(tc.tile_pool(name="ops", bufs=2, space="PSUM"))

        # ---- loads are issued just-in-time, one couple ahead ----
        wqkv_sbs, wo_sbs, x_sbs = [], [], []
        kh = KC // 2

        def issue_couple_loads(g, fine):
            wqkv_sb = wqkv_pool.tile([128, WQK + WVW], bf16, name=f"wqkv_{g}", tag="wqkv")
            wo_sb = wo_pool.tile([128, dm], bf16, name=f"wo_{g}", tag="wo")
            if fine:
                # first couple: per-chunk pieces in consumption order so the
                # qk kc-loop never outruns the load stream
                nc.sync.dma_start(wqkv_sb[:, :2 * 2 * dh], wqkv[g, :, :2 * 2 * dh])
                xts = []
                for e in (0, 1):
                    p = 2 * g + e
                    xtile = xt_pool.tile([128, KC * s_len], bf16, name=f"x_{p}", tag="x")
                    xts.append(xtile)
                    x_sbs.append(xtile)
                # pair 0 on the sync queue, pair 1 on the gpsimd queue --
                # two DMA channels halve the cold-start load time
                XQ = {0: nc.sync, 1: nc.gpsimd}
                for e in (0, 1):
                    XQ[e].dma_start(xts[e][:, :512], xt[2 * g + e, :, :512])
                # bulk weight piece rides the gpsimd queue so it doesn't
                # delay pair-0's x chunks on the sync queue
                nc.gpsimd.dma_start(wqkv_sb[:, 2 * 2 * dh:], wqkv[g, :, 2 * 2 * dh:])
                for e in (0, 1):
                    XQ[e].dma_start(xts[e][:, 512:s_len], xt[2 * g + e, :, 512:s_len])
                for kc in range(1, KC):
                    for e in (0, 1):
                        XQ[e].dma_start(
                            xts[e][:, kc * s_len:(kc + 1) * s_len],
                            xt[2 * g + e, :, kc * s_len:(kc + 1) * s_len])
                nc.sync.dma_start(wo_sb[:], wo[g])
            else:
                nc.sync.dma_start(wqkv_sb[:], wqkv[g])
                for e in (0, 1):
                    p = 2 * g + e
                    xtile = xt_pool.tile([128, KC * s_len], bf16, name=f"x_{p}", tag="x")
                    nc.sync.dma_start(xtile[:, :kh * s_len], xt[p, :, :kh * s_len])
                    nc.sync.dma_start(xtile[:, kh * s_len:], xt[p, :, kh * s_len:])
                    x_sbs.append(xtile)
                nc.sync.dma_start(wo_sb[:], wo[g])
            wqkv_sbs.append(wqkv_sb)
            wo_sbs.append(wo_sb)

        # ident loads FIRST (tiny) so the HAM warm-up burst below can start
        # immediately; the real loads follow on the same queue
        ident_sb = const_pool.tile([128, 128], bf16, name="ident_sb")
        nc.sync.dma_start(ident_sb[:], ident[:, :])
        issue_couple_loads(0, fine=True)
        if n_couples > 1:
            issue_couple_loads(1, fine=False)

        # HAM warm-up/keep-warm fillers: the PE clock-gate defaults to 4/8
        # (1.2 GHz) and only releases to 8/8 after ~3.4us of sustained busy;
        # any ~us idle re-throttles.  The first couple is DMA-load-bound, so
        # dummy ident matmuls are sprinkled at priorities BETWEEN the real
        # work: the list scheduler only runs them when nothing else is ready,
        # keeping the PE array busy through load stalls.
        wu_ps = ops_pool.tile([128, 128], f32, name="warm_ps", tag="ops")

        def warm_fill(n):
            for _ in range(n):
                nc.tensor.matmul(wu_ps[:, 0:128], lhsT=ident_sb[:, :],
                                 rhs=ident_sb[:, :], start=True, stop=True,
                                 skip_group_check=True)

        # 60 matmuls x ~107ns bridges from DMA-queue boot (~3us) to the
        # first data-ready qk matmuls (~9.5us) with no idle window between
        warm_fill(60)

        # engine round-robin for out-copies (psum readers: DVE/ScalarE only);
        # 5:3 DVE:Sc because ScalarE also carries all the exps
        OUT_ENGS = [nc.vector, nc.scalar, nc.vector, nc.scalar,
                    nc.vector, nc.vector, nc.scalar, nc.vector]
        out_rr = [0]

        pending_out = []
        final_split = [False]

        def emit_one_pending():
            if pending_out:
                pending_out.pop(0)()

        def make_out(p, j, zT2_sb, recip_sb, wo_sb):
            """Out-projection for strip-couple (j, j+1): one merged 128-row
            ldweights of zT2 block m, row-packed dj matmuls."""
            m = j // 2
            e = p & 1

            def emit():
                o_sb = osb_pool.tile([128, 2 * dm], bf16, name=f"osb_{p}_{j}", tag="osb")
                for c0 in range(0, dm, MMB):
                    c1 = min(c0 + MMB, dm)
                    o_tiles = []
                    for dj in (0, 1):
                        o_ps = ops_pool.tile([128, 512], f32, name=f"ops_{p}_{j + dj}_{c0}", tag="ops")
                        nc.tensor.matmul(
                            o_ps[:, 0:c1 - c0],
                            lhsT=zT2_sb[64 * dj:64 * dj + dh,
                                        m * 128:(m + 1) * 128],
                            rhs=wo_sb[64 * dj:64 * dj + dh, c0:c1],
                            start=True, stop=True,
                        )
                        o_tiles.append(o_ps)
                    for dj in (0, 1):
                        dst = o_sb[:, dj * dm + c0:dj * dm + c1]
                        osrc = o_tiles[dj][:, 0:c1 - c0]
                        scal = recip_sb[:, e * 4 + (j + dj) % 4:e * 4 + (j + dj) % 4 + 1]
                        eng = OUT_ENGS[out_rr[0] % len(OUT_ENGS)]
                        out_rr[0] += 1
                        if eng is nc.scalar:
                            nc.scalar.mul(dst, osrc, scal)
                        else:
                            eng.tensor_scalar_mul(dst, osrc, scal)
                if final_split[0]:
                    # tail: halve store latency across two queues
                    nc.sync.dma_start(out[p, m][:, :dm], o_sb[:, :dm])
                    nc.gpsimd.dma_start(out[p, m][:, dm:], o_sb[:, dm:])
                else:
                    nc.sync.dma_start(out[p, m], o_sb[:])
            return emit

        for g in range(n_couples):
            if g + 2 < n_couples:
                issue_couple_loads(g + 2, fine=False)
            p0, p1 = 2 * g, 2 * g + 1
            x0, x1 = x_sbs[p0], x_sbs[p1]
            wqkv_sb = wqkv_sbs[g]
            wo_sb = wo_sbs[g]
            last = g == n_couples - 1

            # ---- qk^T projections into 1-bank granules, shared stationary ----
            qk_gr = {}
            for e in (0, 1):
                for hh in (0, 1):
                    qk_gr[e, hh] = acc_pool.tile(
                        [128, 512], f32, name=f"qkps_{2 * g + e}_{hh}", tag="acc")
            # kc-outer: the 4 granule matmuls of each kc share one stationary
            # (single ldweights after dedup); granule-ring slot release gives
            # the previous couple's z extraction enough slack at the boundary
            # (a 2-chunk pair-1 stagger was tried: 164.1us vs 161-163, no win)
            for kc in range(KC):
                for e, xtile in ((0, x0), (1, x1)):
                    for hh in (0, 1):
                        n0 = hh * 512
                        nc.tensor.matmul(
                            qk_gr[e, hh][:, 0:512],
                            lhsT=wqkv_sb[:, kc * 2 * dh:(kc + 1) * 2 * dh],
                            rhs=xtile[:, kc * s_len + n0:kc * s_len + n0 + 512],
                            start=(kc == 0), stop=(kc == KC - 1),
                            skip_group_check=(e == 1 or hh == 1),
                        )
            qkTs, qdups = [], []
            for e, p in ((0, p0), (1, p1)):
                qkT_sb = qkT_pool.tile([128, s_len], bf16, name=f"qkT_{p}", tag="qkT")
                # lo/hi on different engines so both copies run concurrently
                if e == 0:
                    nc.scalar.copy(qkT_sb[:, 0:512], qk_gr[e, 0][:, 0:512])
                    nc.vector.tensor_copy(qkT_sb[:, 512:1024], qk_gr[e, 1][:, 0:512])
                else:
                    nc.vector.tensor_copy(qkT_sb[:, 0:512], qk_gr[e, 0][:, 0:512])
                    nc.scalar.copy(qkT_sb[:, 512:1024], qk_gr[e, 1][:, 0:512])
                # partition swap: swap[0:64]=k^T, swap[64:128]=q^T -- two
                # contiguous [64,1024] DMAs (cheap descriptor count)
                swap_sb = qdup_pool.tile([128, s_len], bf16, name=f"swap_{p}", tag="qdup")
                nc.gpsimd.dma_start(swap_sb[0:dh, :], qkT_sb[dh:2 * dh, :])
                nc.gpsimd.dma_start(swap_sb[dh:2 * dh, :], qkT_sb[0:dh, :])
                qkTs.append(qkT_sb)
                qdups.append(swap_sb)
                emit_one_pending()

            # ---- v^T projections, column-packed via duplicated weights ----
            vt_gr = [scps.tile([128, 512], f32, name=f"vtps_{g}_{hh}", tag="scps")
                     for hh in (0, 1)]
            for kc in range(KC):
                for hh in (0, 1):
                    n0 = hh * 512
                    for e, xtile in ((0, x0), (1, x1)):
                        nc.tensor.matmul(
                            vt_gr[hh][64 * e:64 * e + dh, 0:512],
                            lhsT=wqkv_sb[:, WVO + kc * 128 + 64 * e:
                                         WVO + kc * 128 + 64 * e + dh],
                            rhs=xtile[:, kc * s_len + n0:kc * s_len + n0 + 512],
                            start=(kc == 0), stop=(kc == KC - 1),
                            skip_group_check=True,
                        )
            vT_sb = vT_pool.tile([128, s_len], bf16, name=f"vT_{g}", tag="vT")
            nc.vector.tensor_copy(vT_sb[:, 0:512], vt_gr[0][:, 0:512])
            nc.scalar.copy(vT_sb[:, 512:], vt_gr[1][:, 0:512])
            emit_one_pending()
            emit_one_pending()

            # bf16 transposes: one merged 128-row transpose-load per t-block
            vtrs = [scps.tile([128, NSQ * dh], bf16, name=f"vtr_{2 * g + e}", tag="scps")
                    for e in (0, 1)]
            for t in range(NSQ):
                for e in (0, 1):
                    nc.tensor.transpose(
                        vtrs[e][:, t * dh:(t + 1) * dh],
                        vT_sb[64 * e:64 * e + dh, t * 128:(t + 1) * 128],
                        ident_sb[64 * e:64 * e + dh, 64 * e:64 * e + dh],
                    )
            vaugs = []
            for e, p in ((0, p0), (1, p1)):
                vaug_sb = vaug_pool.tile([128, NSQ * (dh + 1)], bf16, name=f"vaug_{p}", tag="vaug")
                if g == 0:
                    # ones columns persist across pool reuse; set once
                    nc.gpsimd.memset(vaug_sb[:], 1.0)
                nc.vector.tensor_copy(
                    vaug_sb[:].rearrange("p (n d) -> p n d", d=dh + 1)[:, :, 0:dh],
                    vtrs[e][:].rearrange("p (n d) -> p n d", d=dh),
                )
                vaugs.append(vaug_sb)

            # ---- phase B: pairs interleaved per strip-pair ----
            z_gr, zT2_sbs = {}, {}
            lf_sbs = {}
            recip_sbs = {}
            for e, p in ((0, p0), (1, p1)):
                zT2_sbs[e] = zT2_pool.tile([128, NM * 128], bf16, name=f"zT2_{p}", tag="zT2")
            # z granules allocated in release-friendly order
            for e, p in ((0, p0), (1, p1)):
                z_gr[e, 0] = acc_pool.tile([128, 512], f32, name=f"zps_{p}_lo", tag="acc")
            for e, p in ((0, p0), (1, p1)):
                z_gr[e, 1] = acc_pool.tile([128, 512], f32, name=f"zps_{p}_hi", tag="acc")
            for hh in (0, 1):
                lf_sbs[hh] = lf_pool.tile([8, 128], bf16, name=f"lf_{g}_{hh}", tag="lf")
                recip_sbs[hh] = recip_pool.tile([128, 8], f32, name=f"recip_{g}_{hh}", tag="recip")
            stage_sbs = {e: stage_pool.tile([1, s_len], bf16, name=f"lstage_{2 * g + e}", tag="stage")
                         for e in (0, 1)}

            for m in range(NM):
                i0 = 2 * m
                blocks0 = _strip_blocks(i0, s_len)
                blocks1 = _strip_blocks(i0 + 1, s_len)
                nblk = max(len(blocks0), len(blocks1))
                sc_tiles = {}
                # row-packed score matmuls for BOTH pairs (k^T strips as
                # stationaries in opposite partition halves)
                for e, p in ((0, p0), (1, p1)):
                    qkT_sb, swap_sb = qkTs[e], qdups[e]
                    for bi in range(nblk):
                        for di, i, blocks in ((0, i0, blocks0), (1, i0 + 1, blocks1)):
                            bj = bi - (nblk - len(blocks))
                            if bj < 0:
                                continue
                            a, b = blocks[bj]
                            sc_ps = scps.tile([128, 512], f32, name=f"sc_{p}_{i}_{a}", tag="scps")
                            if di == 0:
                                lhsT = swap_sb[0:dh, i * 128:(i + 1) * 128]
                                rhs = qkT_sb[0:dh, a:b]
                            else:
                                lhsT = qkT_sb[dh:2 * dh, i * 128:(i + 1) * 128]
                                rhs = swap_sb[dh:2 * dh, a:b]
                            nc.tensor.matmul(
                                sc_ps[:, 0:b - a], lhsT=lhsT, rhs=rhs,
                                start=True, stop=True,
                            )
                            sc_tiles[(e, i, a)] = sc_ps

                # PE gap fillers: deferred out-couples run here
                emit_one_pending()
                emit_one_pending()

                # exp (ScalarE), diag mask (GpSimd), z matmuls, extraction
                for e, p in ((0, p0), (1, p1)):
                    vaug_sb = vaugs[e]
                    zT2_sb = zT2_sbs[e]
                    for di, i, blocks in ((0, i0, blocks0), (1, i0 + 1, blocks1)):
                        # all exps of the strip first, then both z matmuls
                        # back-to-back so the vaug stationary loads once
                        pts = []
                        for (a, b) in blocks:
                            sc_ps = sc_tiles[(e, i, a)]
                            pt_sb = pstrip_pool.tile([128, 512], bf16, name=f"pt_{p}_{i}_{a}", tag="pstrip")
                            nc.scalar.activation(pt_sb[:, 0:b - a], sc_ps[:, 0:b - a], Exp)
                            if a == 128 * i:  # leading block holds the diag triangle
                                dst = pt_sb[:, 0:128]
                                nc.gpsimd.affine_select(
                                    out=dst, in_=dst,
                                    compare_op=mybir.AluOpType.is_ge,
                                    fill=0.0, base=0,
                                    pattern=[[1, 128]], channel_multiplier=-1,
                                )
                            pts.append(pt_sb)
                        for (a, b), pt_sb in zip(blocks, pts):
                            # z dst granule(s): blocks never straddle col 512
                            gr = z_gr[e, 0] if b <= 512 else z_gr[e, 1]
                            goff = 0 if b <= 512 else 512
                            nc.tensor.matmul(
                                gr[0:dh + 1, a - goff:b - goff],
                                lhsT=vaug_sb[:, i * (dh + 1):(i + 1) * (dh + 1)],
                                rhs=pt_sb[:, 0:b - a],
                                start=(i == 0), stop=(i == (b - 1) // 128),
                                skip_group_check=True,
                            )

                    # eager extraction: even strips -> zT2 top half, odd ->
                    # bottom (DVE psum partitions 0:64 -> sbuf 64:128 is a
                    # legal 32-aligned shift).  Non-last couples defer to one
                    # strided-gather copy per half (half the instruction
                    # overhead); the last couple stays per-iteration (lag 1).
                    hh = m // 2
                    gr = z_gr[e, 0] if m < 2 else z_gr[e, 1]
                    c0 = 256 * m - (0 if m < 2 else 512)
                    if last:
                        nc.vector.tensor_copy(
                            zT2_sb[0:dh, m * 128:(m + 1) * 128], gr[0:dh, c0:c0 + 128])
                        nc.vector.tensor_copy(
                            zT2_sb[dh:2 * dh, m * 128:(m + 1) * 128], gr[0:dh, c0 + 128:c0 + 256])
                        nc.vector.tensor_copy(
                            stage_sbs[e][0:1, 256 * m:256 * m + 256], gr[dh:dh + 1, c0:c0 + 256])
                    elif m in (1, 3):
                        hv = gr[:, 0:512].rearrange("p (t o b) -> p t o b", o=2, b=128)
                        zv = zT2_sb[:, (m - 1) * 128:(m + 1) * 128].rearrange(
                            "p (t b) -> p t b", b=128)
                        nc.vector.tensor_copy(zv[0:dh], hv[0:dh, :, 0, :])
                        nc.vector.tensor_copy(zv[dh:2 * dh], hv[0:dh, :, 1, :])
                        nc.vector.tensor_copy(
                            stage_sbs[e][0:1, 512 * hh:512 * (hh + 1)],
                            gr[dh:dh + 1, 0:512])

                # l transpose: one 8-row matmul per couple-half gives both
                # pairs' denominator columns; last couple runs it per
                # iteration (lag-1) to shrink the final flush
                if m in (1, 3) or last:
                    hh = m // 2
                    lf_sb, recip_sb = lf_sbs[hh], recip_sbs[hh]
                    for e in (0, 1):
                        if last:
                            r0 = e * 4 + (2 * m) % 4
                            nc.gpsimd.dma_start(
                                lf_sb[r0:r0 + 2, 0:128],
                                stage_sbs[e][0:1, 256 * m:256 * (m + 1)])
                        else:
                            nc.gpsimd.dma_start(
                                lf_sb[e * 4:e * 4 + 4, 0:128],
                                stage_sbs[e][0:1, 512 * hh:512 * (hh + 1)])
                    l_ps = ops_pool.tile([128, 8], f32, name=f"lps_{g}_{m}", tag="ops")
                    nc.tensor.matmul(
                        l_ps[:, 0:8], lhsT=lf_sb[0:8, 0:128],
                        rhs=ident_sb[0:8, 0:8], start=True, stop=True,
                    )
                    nc.vector.reciprocal(recip_sb[:, 0:8], l_ps[:, 0:8])
                    if last:
                        for e, p in ((0, p0), (1, p1)):
                            pending_out.append(make_out(p, 2 * m, zT2_sbs[e], recip_sb, wo_sb))
                    else:
                        for e, p in ((0, p0), (1, p1)):
                            pending_out.append(make_out(p, 2 * m - 2, zT2_sbs[e], recip_sb, wo_sb))
                            pending_out.append(make_out(p, 2 * m, zT2_sbs[e], recip_sb, wo_sb))

        final_split[0] = True
        while pending_out:
            emit_one_pending()

    nc.finalize()
    _optimize_ldweights(nc, mybir)
    return nc


def _optimize_ldweights(nc, mybir):
    """Merge and dedup PE Ldweights on the final scheduled stream.

    bacc lowers every matmul to an Ldweights+Matmult pair, walrus runs with
    --enable-ldw-opt=false, and ldweights streams at ~1.2 GHz with cost
    proportional to its COLUMN count only -- a 128-row load costs the same
    as a 64-row one.  Two rewrites:

    1. pair-merge: a row-packed pair loads vertical halves of the same tile
       at the same columns (kkT/zT2/vT layouts are built for this), and the
       duplicated-wv layout loads horizontal halves of one 128-col chunk.
       The earlier load's AP is widened to cover both and the later load is
       deleted (its waits join the merged load: both halves' producers must
       be done before the single load streams).
    2. cover-dedup: a load is dropped when the array region it would fill
       already holds identical data (repeat stationaries across moving
       blocks); its waits move onto the next PE instruction.

    Merging is restricted to a tensor-name safe-list whose producers never
    depend on PE progress between the pair (no wait-cycle risk)."""
    pe = mybir.EngineType.PE
    MERGE_OK = {"kkT", "zT2", "vT", "wqkv"}
    removed = 0

    def parse(inst):
        ap = inst.ins[0]
        a = [list(x) for x in ap.ap]
        if len(a) != 2 or a[1][0] != 1:
            return None
        (pstride, np_), (_, ncols) = a
        if pstride <= 0:
            return None
        pbase, coff = divmod(ap.offset, pstride)
        tp = inst.tile_position or (0, 0)
        return dict(mem=ap.memref, dt=repr(ap.dtype), np=np_, ncols=ncols,
                    pstride=pstride, pbase=pbase, coff=coff, tp=tp,
                    it=repr(inst.is_transpose), pm=repr(inst.perf_mode))

    def region(d):
        return (d["tp"][0], d["tp"][0] + d["np"], d["tp"][1], d["tp"][1] + d["ncols"])

    def overlaps(a, b):
        return a[0] < b[1] and b[0] < a[1] and a[2] < b[3] and b[2] < a[3]

    def covers(k, c):
        if (k["mem"] != c["mem"] or k["dt"] != c["dt"] or k["it"] != c["it"]
                or k["pm"] != c["pm"]):
            return False
        if k["tp"][0] - k["pbase"] != c["tp"][0] - c["pbase"]:
            return False
        if k["tp"][1] - k["coff"] != c["tp"][1] - c["coff"]:
            return False
        rk, rc = region(k), region(c)
        return rk[0] <= rc[0] and rc[1] <= rk[1] and rk[2] <= rc[2] and rc[3] <= rk[3]

    def same_meta(x, y):
        return (x["mem"] == y["mem"] and x["dt"] == y["dt"] and x["it"] == y["it"]
                and x["pm"] == y["pm"] and x["pstride"] == y["pstride"])

    def pair_kind(x, y):
        """row: vertical halves (top at (0,c), bottom at (64,c), same cols);
        col: horizontal halves of one 128-col chunk at (0,0)/(0,64)."""
        if not same_meta(x, y) or x["mem"].split("_")[0] not in MERGE_OK:
            return None
        if (x["np"] == 64 and y["np"] == 64 and x["ncols"] == y["ncols"]
                and x["coff"] == y["coff"] and x["tp"][1] == y["tp"][1]
                and {(x["pbase"], x["tp"][0]), (y["pbase"], y["tp"][0])}
                == {(0, 0), (64, 64)}):
            return "row"
        if (x["np"] == 128 and y["np"] == 128 and x["ncols"] == 64 and y["ncols"] == 64
                and x["pbase"] == 0 and y["pbase"] == 0
                and {(x["coff"] - min(x["coff"], y["coff"]), x["tp"][1]),
                     (y["coff"] - min(x["coff"], y["coff"]), y["tp"][1])}
                == {(0, 0), (64, 64)}):
            return "col"
        return None

    def apply_merge(inst, x, y, kind):
        ap = inst.ins[0]
        if kind == "row":
            ap.offset = x["coff"]
            ap.ap = [[x["pstride"], 128], [1, x["ncols"]]]
            inst.tile_position = (0, x["tp"][1])
            inst.tile_size = (128, x["ncols"])
            return dict(x, np=128, pbase=0, coff=x["coff"], tp=(0, x["tp"][1]))
        else:
            c0 = min(x["coff"], y["coff"])
            ap.offset = c0
            ap.ap = [[x["pstride"], 128], [1, 128]]
            inst.tile_position = (0, 0)
            inst.tile_size = (128, 128)
            return dict(x, ncols=128, coff=c0, tp=(0, 0))

    MAX_WAITS = 2

    def dedup_waits(waits):
        """Collapse same-semaphore ge-imm waits to the max value."""
        out = []
        best = {}
        for w in waits:
            if getattr(w, "wait_mode", None) == "sem-ge-imm" and w.wait_reg is None:
                key = (w.sync_type, w.id)
                cur = best.get(key)
                if cur is None:
                    best[key] = w
                    out.append(w)
                elif w.wait_value > cur.wait_value:
                    out[out.index(cur)] = w
                    best[key] = w
            else:
                out.append(w)
        return out

    def try_merge_sync(dst_inst, src_inst):
        """Combine src's waits/updates into dst; False if over the ISA cap."""
        ssi = src_inst.sync_info
        dsi = dst_inst.sync_info
        waits = dedup_waits(
            (list(dsi.on_wait) if dsi else []) + (list(ssi.on_wait) if ssi else []))
        upds = (list(dsi.on_update) if dsi else []) + (list(ssi.on_update) if ssi else [])
        if len(waits) > MAX_WAITS:
            return False
        if dsi is None:
            dst_inst.sync_info = mybir.SyncInfo(on_wait=waits, on_update=upds)
        else:
            dsi.on_wait = waits
            dsi.on_update = upds
        return True

    ES = getattr(mybir, "InstEventSemaphore", ())

    for fn in nc.m.functions:
        for blk in fn.blocks:
            insts = blk.instructions
            n = len(insts)
            pe_idx = [i for i in range(n)
                      if getattr(insts[i], "engine", None) == pe]
            drop = set()

            # ---- pass 1: pair-merge on the PE stream ----
            k = 0
            while k < len(pe_idx):
                i = pe_idx[k]
                inst = insts[i]
                if i in drop or not isinstance(inst, mybir.InstLdweights):
                    k += 1
                    continue
                x = parse(inst)
                if x is None or x["mem"].split("_")[0] not in MERGE_OK:
                    k += 1
                    continue
                # scan forward for the partner: MMs/event-sems may intervene
                j = k + 1
                partner = None
                mid_mms = []
                while j < len(pe_idx):
                    inst2 = insts[pe_idx[j]]
                    if isinstance(inst2, mybir.InstLdweights):
                        y = parse(inst2)
                        if y is not None and pair_kind(x, y):
                            partner = (pe_idx[j], inst2, y)
                        break
                    if isinstance(inst2, mybir.InstMatmult):
                        mid_mms.append(inst2)
                        j += 1
                        continue
                    if ES and isinstance(inst2, ES):
                        j += 1
                        continue
                    break
                if partner is None:
                    k += 1
                    continue
                jj, y_inst, y = partner
                # the partner's region gets loaded earlier than before: no
                # intervening matmul may be streaming through it
                yreg = region(y) if pair_kind(x, y) == "row" else (
                    0, 128, y["tp"][1], y["tp"][1] + y["ncols"])
                bad = False
                for mm in mid_mms:
                    tp = mm.tile_position or (0, 0)
                    ts = mm.tile_size or (128, 128)
                    if overlaps((tp[0], tp[0] + ts[0], tp[1], tp[1] + ts[1]), yreg):
                        bad = True
                        break
                if bad or not try_merge_sync(inst, y_inst):
                    k += 1
                    continue
                apply_merge(inst, x, y, pair_kind(x, y))
                drop.add(jj)
                removed += 1
                k += 1

            # ---- pass 2: cover-dedup; waits forward to the next PE inst ----
            live = [ii for ii in pe_idx if ii not in drop]
            kept = []
            for pos, ii in enumerate(live):
                inst = insts[ii]
                if isinstance(inst, mybir.InstLdweights):
                    d = parse(inst)
                    si = inst.sync_info
                    no_upd = si is None or not si.on_update
                    if d is not None and no_upd and any(covers(kk, d) for kk in kept):
                        ok = True
                        if si is not None and si.on_wait:
                            ok = (pos + 1 < len(live)
                                  and try_merge_sync(insts[live[pos + 1]], inst))
                        if ok:
                            removed += 1
                            drop.add(ii)
                            continue
                    if d is not None:
                        kept = [kk for kk in kept if not overlaps(region(kk), region(d))]
                        kept.append(d)
                    else:
                        kept = []
                elif isinstance(inst, mybir.InstMatmult):
                    pass
                elif ES and isinstance(inst, ES):
                    pass
                else:
                    kept = []

            if drop:
                keep = [insts[i] for i in range(n) if i not in drop]
                del blk.instructions[:]
                for inst in keep:
                    blk.instructions.append(inst)
    return removed


def prepare_shards(normalized_resid_pre, W_Q, b_Q, W_K, b_K, W_V, b_V, W_O, b_O):
    """Host-side layout: returns in_maps for the 8 cores."""
    x = np.asarray(normalized_resid_pre, dtype=np.float32)
    scale = 1.0 / np.sqrt(DH)
    KC = DM // 128

    pair_map = _core_pair_map()

    # x^T per (core, slot), partition-major: [128, KC*S]
    xt_f = x.transpose(0, 2, 3, 1)  # [B, H, DM, S]
    # W_Q pre-scaled by 1/sqrt(DH) so scores come out pre-scaled
    wqk_h = np.concatenate([np.asarray(W_Q) * scale, np.asarray(W_K)], axis=-1)
    wv_h = np.asarray(W_V)  # [H, DM, DH]
    wo_h = np.asarray(W_O)  # [H, DH, DM]

    ident = np.eye(128).astype(BF16)

    in_maps = []
    for c in range(N_CORES):
        xts, wqkvs, wos = [], [], []
        for s in range(PPC):
            b, h = pair_map[c][s]
            xts.append(
                xt_f[b, h].reshape(KC, 128, S).transpose(1, 0, 2).reshape(128, KC * S))
            if s % 2 == 0:
                wv_c = wv_h[h].reshape(KC, 128, DH).transpose(1, 0, 2)  # [128, KC, DH]
                wv_dup = np.concatenate([wv_c, wv_c], axis=2)  # [128, KC, 128]
                wqkvs.append(np.concatenate(
                    [wqk_h[h].reshape(KC, 128, 2 * DH).transpose(1, 0, 2).reshape(128, KC * 2 * DH),
                     wv_dup.reshape(128, KC * 128)],
                    axis=1))
                wos.append(np.concatenate([wo_h[h], wo_h[h]], axis=0))  # [128, DM]
        in_maps.append({
            "xt": np.ascontiguousarray(np.stack(xts)).astype(BF16),
            "wqkv": np.ascontiguousarray(np.stack(wqkvs)).astype(BF16),
            "wo": np.ascontiguousarray(np.stack(wos)).astype(BF16),
            "ident": ident,
        })
    return in_maps


def _ensure_profile_hook():
    """The agent image lacks ``antenv.axon_hooks``; shim it and install the
    ctypes NTFF hook from trn_boot so trace=True works under axon."""
    import importlib
    import sys
    import types
    try:
        importlib.import_module("antenv.axon_hooks")
        return True
    except ImportError:
        pass
    try:
        import antenv
        mod = types.ModuleType("antenv.axon_hooks")
        _state = {"hook": None}
        mod.set_axon_ntff_profile_hook = lambda h: _state.__setitem__("hook", h)
        mod.get_axon_ntff_profile_hook = lambda: _state["hook"]
        sys.modules["antenv.axon_hooks"] = mod
        antenv.axon_hooks = mod
        from trn_agent_boot.trn_boot import _ntff_profile_via_ctypes
        hook = _ntff_profile_via_ctypes("/opt/axon/libaxon_pjrt.so")
        if hook is not None:
            mod.set_axon_ntff_profile_hook(hook)
        return hook is not None
    except Exception:
        return False


def kernel(**inputs):
    global LAST_EXEC_TIME_NS, LAST_RESULTS
    from concourse.bass_utils import run_bass_kernel_spmd

    in_maps = prepare_shards(**inputs)
    nc = build_nc()

    trace = bool(int(os.environ.get("KERNEL_PROFILE", "0")))
    tmpdir = None
    if trace:
        trace = _ensure_profile_hook()
        if trace:
            tmpdir = os.environ.get("KERNEL_PROFILE_DIR") or None
    res = run_bass_kernel_spmd(nc, in_maps, list(range(N_CORES)), trace=trace,
                               tmpdir=tmpdir)
    LAST_EXEC_TIME_NS = res.exec_time_ns
    LAST_RESULTS = res

    pair_map = _core_pair_map()
    out = np.empty((B, S, H, DM), dtype=np.float32)
    for c in range(N_CORES):
        dev = np.asarray(res.results[c]["out"], dtype=np.float32)
        # [PPC, S//256, 128, 2*DM] -> [PPC, S, DM]
        dev = (dev.reshape(PPC, S // 256, 128, 2, DM)
               .transpose(0, 1, 3, 2, 4).reshape(PPC, S, DM))
        for s in range(PPC):
            b, h = pair_map[c][s]
            out[b, :, h, :] = dev[s]

    b_O = np.asarray(inputs["b_O"], dtype=np.float32)
    b_V = np.asarray(inputs["b_V"], dtype=np.float32)
    b_Q = np.asarray(inputs["b_Q"], dtype=np.float32)
    b_K = np.asarray(inputs["b_K"], dtype=np.float32)
    if np.any(b_Q) or np.any(b_K):
        raise NotImplementedError("nonzero b_Q/b_K not supported by this kernel")
    extra = b_O[None, :] / H  # [1, DM] broadcast over heads
    if np.any(b_V):
        extra = extra + np.einsum(
            "hd,hdm->hm", b_V, np.asarray(inputs["W_O"], dtype=np.float32))
    if np.any(extra):
        out = out + extra[None, None]
    return np.ascontiguousarray(out, dtype=np.float32)
